# revision 1
# baseline (speedup 1.0000x reference)
# Multi-head attention (B=2, T=2048, D=1024, H=16) on 8 TRN2 NeuronCores.
#
# Sharding: tensor-parallel over heads. Each core owns 2 heads (a 128-wide
# slice of the hidden dim): it computes its q/k/v projection slice, full
# attention for its 4 (batch, head) pairs, and a partial output projection
# over its slice of the contraction. The 8 partial outputs are summed on the
# host (the TP all-reduce, done as part of unsharding), plus the output bias.
#
# Layouts (per core):
#   xT   [D=1024, B*T=4096]   x transposed so the contraction dim is on SBUF
#                             partitions for the projection matmuls.
#   qT/kT [128, 4096]         head-dim on partitions (2 heads stacked), token
#                             on free axis -> S^T tiles come out of the PE
#                             directly with softmax's reduction on the free
#                             axis of P^T's consumer.
#   v    [128tok, 32kt, 2h, 65]  natural [token, dim] layout per k-tile with a
#                             ones column appended: the ctx matmul then yields
#                             the softmax denominator for free in row 64.
#   ctxT [64, 2h, 4096]       per-head contraction layout for the output
#                             projection (K=64 accumulation over both heads).
import sys
import types

import numpy as np


def _install_ntff_hook_shim():
    """This image's `antenv` lacks `axon_hooks`, which bass_utils imports
    unconditionally when tracing is requested (e.g. BASS_TRACE=1). Provide
    the module and register the ctypes NTFF hook the way trn_boot would."""
    try:
        import antenv.axon_hooks  # noqa: F401

        return
    except ImportError:
        pass
    try:
        import antenv
    except ImportError:
        return
    mod = types.ModuleType("antenv.axon_hooks")
    _h = [None]
    mod.set_axon_ntff_profile_hook = lambda h: _h.__setitem__(0, h)
    mod.get_axon_ntff_profile_hook = lambda: _h[0]
    sys.modules["antenv.axon_hooks"] = mod
    antenv.axon_hooks = mod
    try:
        from trn_agent_boot.trn_boot import _ntff_profile_via_ctypes

        mod.set_axon_ntff_profile_hook(
            _ntff_profile_via_ctypes("/opt/axon/libaxon_pjrt.so")
        )
    except Exception:
        pass


_install_ntff_hook_shim()

import concourse.bass as bass
import concourse.mybir as mybir
from concourse.bass_utils import run_bass_kernel_spmd
from concourse.masks import make_identity
from concourse.tile import TileContext

B, T, D, H = 2, 2048, 1024, 16
HD = D // H          # 64
NCORES = 8
BT = B * T           # 4096
E = D // NCORES      # 128 = per-core slice of hidden dim (2 heads)
HPC = E // HD        # 2 heads per core

F32 = mybir.dt.float32
F32R = mybir.dt.float32r
AF = mybir.ActivationFunctionType

# Matmul compute dtype: float32r streams fp32 data through the PE at full
# (bf16) rate with relaxed multiply precision. Set to F32 for exact-but-4x-
# slower matmuls.
MM_DT = F32R

TCH = 512            # token chunk for projections / q chunks
NTCH = BT // TCH     # 8
NKT = BT // 128      # 32 token tiles of 128
KTB = T // 128       # 16 k-tiles per batch


def build_nc():
    nc = bass.Bass()

    xT = nc.dram_tensor("xT", [D, BT], MM_DT, kind="ExternalInput")
    wqT = nc.dram_tensor("wqT", [D, E], MM_DT, kind="ExternalInput")
    wkT = nc.dram_tensor("wkT", [D, E], MM_DT, kind="ExternalInput")
    wvT = nc.dram_tensor("wvT", [D, E], MM_DT, kind="ExternalInput")
    bq = nc.dram_tensor("bq", [E, 1], F32, kind="ExternalInput")
    bk = nc.dram_tensor("bk", [E, 1], F32, kind="ExternalInput")
    bv = nc.dram_tensor("bv", [E, 1], F32, kind="ExternalInput")
    wo = nc.dram_tensor("wo", [HD, HPC, D], MM_DT, kind="ExternalInput")
    ones64 = nc.dram_tensor("ones64", [128, HD], MM_DT, kind="ExternalInput")
    out = nc.dram_tensor("out", [BT, D], F32, kind="ExternalOutput")

    with TileContext(nc) as tc:
        with (
            nc.allow_low_precision(reason="float32r is deliberate (matmul speed)"),
            tc.tile_pool(name="const", bufs=1) as cpool,
            tc.tile_pool(name="pers", bufs=1) as pers,
            tc.tile_pool(name="work", bufs=2) as work,
            tc.tile_pool(name="psum", bufs=2, space="PSUM") as psum,
        ):
            # ---- constants -------------------------------------------------
            wq_sb = cpool.tile([128, D // 128, E], MM_DT, name="wq_sb")
            wk_sb = cpool.tile([128, D // 128, E], MM_DT, name="wk_sb")
            wv_sb = cpool.tile([128, D // 128, E], MM_DT, name="wv_sb")
            nc.sync.dma_start(wq_sb, wqT.rearrange("(n p) m -> p n m", p=128))
            nc.sync.dma_start(wk_sb, wkT.rearrange("(n p) m -> p n m", p=128))
            nc.sync.dma_start(wv_sb, wvT.rearrange("(n p) m -> p n m", p=128))
            wo_sb = cpool.tile([HD, HPC, D], MM_DT, name="wo_sb")
            nc.sync.dma_start(wo_sb, wo[:, :, :])
            bq_sb = cpool.tile([E, 1], F32, name="bq_sb")
            bk_sb = cpool.tile([E, 1], F32, name="bk_sb")
            bv_sb = cpool.tile([E, 1], F32, name="bv_sb")
            nc.sync.dma_start(bq_sb, bq[:, :])
            nc.sync.dma_start(bk_sb, bk[:, :])
            nc.sync.dma_start(bv_sb, bv[:, :])
            ident = cpool.tile([128, 128], F32, name="ident")
            make_identity(nc, ident)
            ones_row = cpool.tile([128, HD], MM_DT, name="ones_row")
            nc.sync.dma_start(ones_row, ones64[:, :])

            # ---- persistent activations -----------------------------------
            qT = pers.tile([E, BT], MM_DT, name="qT")
            kT = pers.tile([E, BT], MM_DT, name="kT")
            v = pers.tile([128, NKT, HPC, HD + 1], MM_DT, name="v")
            ctxT = pers.tile([HD, HPC, BT], MM_DT, name="ctxT")
            nc.sync.dma_start(
                v[:, :, :, HD], ones64[:, : NKT * HPC]
            )

            # ---- phase A: QKV projections ---------------------------------
            for t in range(NTCH):
                cols = bass.ts(t, TCH)
                xt = work.tile([128, D // 128, TCH], MM_DT, name="xt", tag="xt", bufs=2)
                nc.sync.dma_start(
                    xt, xT[:, cols].rearrange("(n p) m -> p n m", p=128)
                )
                for w_sb, b_sb, dst in (
                    (wq_sb, bq_sb, qT),
                    (wk_sb, bk_sb, kT),
                    (wv_sb, bv_sb, None),
                ):
                    ps = psum.tile([128, TCH], F32, name="ps_mm", tag="mm", bufs=2)
                    for d in range(D // 128):
                        nc.tensor.matmul(
                            ps,
                            lhsT=w_sb[:, d, :],
                            rhs=xt[:, d, :],
                            start=(d == 0),
                            stop=(d == D // 128 - 1),
                        )
                    if dst is not None:
                        nc.scalar.activation(
                            dst[:, cols], ps, AF.Identity, bias=b_sb, scale=1.0
                        )
                    else:
                        vt = work.tile([128, TCH], F32, name="vt", tag="vt", bufs=2)
                        nc.scalar.activation(vt, ps, AF.Identity, bias=b_sb, scale=1.0)
                        # transpose v back to [token, dim] layout, 128 at a time
                        for i in range(TCH // 128):
                            kt_idx = t * (TCH // 128) + i
                            tp = psum.tile(
                                [128, 128], F32, name="tp", tag="s", bufs=2
                            )
                            nc.tensor.transpose(tp, vt[:, bass.ts(i, 128)], ident)
                            for h in range(HPC):
                                nc.vector.tensor_copy(
                                    v[:, kt_idx, h, 0:HD], tp[:, bass.ts(h, HD)]
                                )

            # ---- phase B: attention (flash-style, per (batch, qchunk)) -----
            # Per k-tile, both heads' S^T matmuls are row-tiled (T0/T8) so
            # they run concurrently on the PE and T8's weight-load overlaps
            # T0's matmul; both land in one [128, 1024] PSUM tile so a single
            # Exp serves both heads. ctx matmuls are software-pipelined one
            # 4-k-tile block behind the S matmuls to keep the PE busy during
            # the exps without thrashing the PE tiling mode per k-tile.
            BLK = 4
            for b in range(B):
                for qc in range(T // TCH):
                    q0 = b * T + qc * TCH
                    cps = []
                    for h in range(HPC):
                        cp = psum.tile(
                            [HD + 1, TCH], F32, name=f"cp{h}", tag=f"ctx{h}", bufs=1
                        )
                        cps.append(cp)
                    pts = {}
                    for blk in range(KTB // BLK + 1):
                        if blk < KTB // BLK:
                            for kt in range(blk * BLK, (blk + 1) * BLK):
                                k0 = b * T + kt * 128
                                sp = psum.tile(
                                    [128, HPC * TCH], F32, name="sp", tag="s", bufs=2
                                )
                                for h in range(HPC):
                                    he = bass.ts(h, HD)
                                    nc.tensor.matmul(
                                        sp[:, bass.ts(h, TCH)],
                                        lhsT=kT[he, k0 : k0 + 128],
                                        rhs=qT[he, q0 : q0 + TCH],
                                        start=True,
                                        stop=True,
                                    )
                                pt = work.tile(
                                    [128, HPC * TCH], MM_DT, name="pt", tag="pt",
                                    bufs=13,
                                )
                                nc.scalar.activation(pt, sp, AF.Exp, scale=1.0 / 8.0)
                                pts[kt] = pt
                        if blk > 0:
                            for kt in range((blk - 1) * BLK, blk * BLK):
                                for h in range(HPC):
                                    nc.tensor.matmul(
                                        cps[h],
                                        lhsT=v[:, b * KTB + kt, h, :],
                                        rhs=pts[kt][:, bass.ts(h, TCH)],
                                        start=(kt == 0),
                                        stop=(kt == KTB - 1),
                                        skip_group_check=True,
                                    )
                    # normalize: ctxT = cp[0:64] * (1 / cp[64]) broadcast
                    for h in range(HPC):
                        cs = work.tile(
                            [HD + 1, TCH], MM_DT, name="cs", tag="cs", bufs=2
                        )
                        nc.vector.tensor_copy(cs, cps[h])
                        nc.vector.reciprocal(
                            cs[HD : HD + 1, :], cs[HD : HD + 1, :]
                        )
                        # broadcast the reciprocal row to all 64 ctx
                        # partitions with a K=1 ones outer-product on the PE
                        # (engines can't shift partitions)
                        rb = psum.tile([HD, TCH], F32, name="rb", tag="mm", bufs=2)
                        nc.tensor.matmul(
                            rb,
                            lhsT=ones_row[HD : HD + 1, :],
                            rhs=cs[HD : HD + 1, :],
                            start=True,
                            stop=True,
                        )
                        nc.vector.tensor_tensor(
                            ctxT[:, h, q0 : q0 + TCH],
                            cs[0:HD, :],
                            rb,
                            op=mybir.AluOpType.mult,
                        )

            # ---- phase C: output projection (partial over this core's slice)
            for tt in range(NKT):
                trows = bass.ts(tt, 128)
                for nch in range(D // TCH):
                    po = psum.tile([128, TCH], F32, name="po", tag="mm", bufs=2)
                    for h in range(HPC):
                        nc.tensor.matmul(
                            po,
                            lhsT=ctxT[:, h, trows],
                            rhs=wo_sb[:, h, bass.ts(nch, TCH)],
                            start=(h == 0),
                            stop=(h == HPC - 1),
                        )
                    ob = work.tile([128, TCH], F32, name="ob", tag="ob", bufs=3)
                    nc.vector.tensor_copy(ob, po)
                    nc.sync.dma_start(out[trows, bass.ts(nch, TCH)], ob)

    _split_matmul_waits(nc)
    return nc


def _split_matmul_waits(nc):
    """This walrus allows only one sync wait per engine instruction (and none
    on fp32/f32r InstMatmult, whose embedded S3_LW carries the wait slot).
    Move excess waits onto InstEventSemaphore instructions (capacity 2)
    inserted just before the owner in the same engine stream — sequencer
    dispatch is in-order, so semantics are unchanged."""
    ctr = 0
    for f in nc.m.functions:
        for blk in f.blocks:
            out = []
            for inst in blk.instructions:
                si = inst.sync_info
                if (
                    si is not None
                    and not isinstance(inst, mybir.InstEventSemaphore)
                    and len(si.on_wait) > 1
                ):
                    waits = list(si.on_wait)
                    keep = [waits.pop(0)]
                    for i in range(0, len(waits), 2):
                        ev = mybir.InstEventSemaphore(name=f"I-exwait-{ctr}")
                        ctr += 1
                        ev.engine = inst.engine
                        ev.sync_info = mybir.SyncInfo(
                            on_wait=waits[i : i + 2], on_update=[]
                        )
                        nc.register_instruction(ev)
                        out.append(ev)
                    si.on_wait = keep
                out.append(inst)
            blk.instructions[:] = out


_CACHE = {}


def _get_nc():
    if "nc" not in _CACHE:
        _CACHE["nc"] = build_nc()
    return _CACHE["nc"]


def make_in_maps(x, w_qkv, b_qkv, w_out):
    x = np.ascontiguousarray(np.asarray(x, np.float32)).reshape(BT, D)
    w_qkv = np.asarray(w_qkv, np.float32)
    b_qkv = np.asarray(b_qkv, np.float32)
    w_out = np.asarray(w_out, np.float32)

    xT = np.ascontiguousarray(x.T)  # [D, BT]
    wq, wk, wv = w_qkv[0:D], w_qkv[D : 2 * D], w_qkv[2 * D : 3 * D]
    bqs, bks, bvs = b_qkv[0:D], b_qkv[D : 2 * D], b_qkv[2 * D : 3 * D]

    in_maps = []
    for c in range(NCORES):
        rs = slice(E * c, E * (c + 1))
        # wo_c[j, h, o] = w_out[o, E*c + h*HD + j]
        wo_c = np.ascontiguousarray(
            w_out[:, rs].T.reshape(HPC, HD, D).transpose(1, 0, 2)
        )
        in_maps.append(
            {
                "xT": xT,
                "wqT": np.ascontiguousarray(wq[rs].T),
                "wkT": np.ascontiguousarray(wk[rs].T),
                "wvT": np.ascontiguousarray(wv[rs].T),
                "bq": np.ascontiguousarray(bqs[rs])[:, None],
                "bk": np.ascontiguousarray(bks[rs])[:, None],
                "bv": np.ascontiguousarray(bvs[rs])[:, None],
                "wo": wo_c,
                "ones64": np.ones((128, HD), np.float32),
            }
        )
    return in_maps


def _combine(results, b_out):
    acc = results[0]["out"].copy()
    for r in results[1:]:
        acc += r["out"]
    acc += np.asarray(b_out, np.float32)[None, :]
    return acc.reshape(B, T, D)


def kernel(x, w_qkv, b_qkv, w_out, b_out):
    in_maps = make_in_maps(x, w_qkv, b_qkv, w_out)
    res = run_bass_kernel_spmd(_get_nc(), in_maps, core_ids=list(range(NCORES)))
    return _combine(res.results, b_out)


def kernel_traced(x, w_qkv, b_qkv, w_out, b_out):
    """Like kernel() but profiles the run; returns (output, exec_time_ns)."""
    in_maps = make_in_maps(x, w_qkv, b_qkv, w_out)
    res = run_bass_kernel_spmd(
        _get_nc(), in_maps, core_ids=list(range(NCORES)), trace=True
    )
    return _combine(res.results, b_out), res.exec_time_ns



# revision 4
# speedup vs baseline: 1.2474x; 1.2474x over previous
# Multi-head attention (B=2, T=2048, D=1024, H=16) on 8 TRN2 NeuronCores.
#
# Sharding: tensor-parallel over heads. Each core owns 2 heads (a 128-wide
# slice of the hidden dim): it computes its q/k/v projection slice, full
# attention for its 4 (batch, head) pairs, and a partial output projection
# over its slice of the contraction. The 8 partial outputs are summed on the
# host (the TP all-reduce, done as part of unsharding), plus the output bias.
#
# All matmul operands are bf16 (PSUM accumulation stays fp32): rel tolerance
# is 2e-2 and bf16 keeps us ~2.5e-3, while halving DMA/SBUF traffic and
# letting weight loads overlap matmul streaming.
#
# Layouts (per core):
#   xT   [D=1024, B*T=4096]   x transposed so the contraction dim is on SBUF
#                             partitions for the projection matmuls.
#   qT/kT [128, 4096]         head-dim on partitions (2 heads stacked), token
#                             on free axis -> S^T tiles come out of the PE
#                             directly with softmax's reduction on the free
#                             axis of P^T's consumer.
#   v    [128tok, 32kt, 2h, 65]  natural [token, dim] layout per k-tile with a
#                             ones column appended: the ctx matmul then yields
#                             the softmax denominator for free in row 64.
#   ctxT [128, 4096]          both heads' normalized context stacked on
#                             partitions (h0 rows 0-63, h1 rows 64-127) so the
#                             output projection contracts K=128 in one matmul
#                             per tile.
import sys
import types

import numpy as np


def _install_ntff_hook_shim():
    """This image's `antenv` lacks `axon_hooks`, which bass_utils imports
    unconditionally when tracing is requested (e.g. BASS_TRACE=1). Provide
    the module and register the ctypes NTFF hook the way trn_boot would."""
    try:
        import antenv.axon_hooks  # noqa: F401

        return
    except ImportError:
        pass
    try:
        import antenv
    except ImportError:
        return
    mod = types.ModuleType("antenv.axon_hooks")
    _h = [None]
    mod.set_axon_ntff_profile_hook = lambda h: _h.__setitem__(0, h)
    mod.get_axon_ntff_profile_hook = lambda: _h[0]
    sys.modules["antenv.axon_hooks"] = mod
    antenv.axon_hooks = mod
    try:
        from trn_agent_boot.trn_boot import _ntff_profile_via_ctypes

        mod.set_axon_ntff_profile_hook(
            _ntff_profile_via_ctypes("/opt/axon/libaxon_pjrt.so")
        )
    except Exception:
        pass


_install_ntff_hook_shim()

import ml_dtypes

import concourse.bass as bass
import concourse.mybir as mybir
from concourse.bass_utils import run_bass_kernel_spmd
from concourse.tile import TileContext

B, T, D, H = 2, 2048, 1024, 16
HD = D // H          # 64
NCORES = 8
BT = B * T           # 4096
E = D // NCORES      # 128 = per-core slice of hidden dim (2 heads)
HPC = E // HD        # 2 heads per core

F32 = mybir.dt.float32
BF = mybir.dt.bfloat16
AF = mybir.ActivationFunctionType

TCH = 512            # token chunk for projections / q chunks
NTCH = BT // TCH     # 8
NKT = BT // 128      # 32 token tiles of 128
KTB = T // 128       # 16 k-tiles per batch


def build_nc():
    nc = bass.Bass()

    xT = nc.dram_tensor("xT", [D, BT], BF, kind="ExternalInput")
    wqT = nc.dram_tensor("wqT", [D, E], BF, kind="ExternalInput")
    wkT = nc.dram_tensor("wkT", [D, E], BF, kind="ExternalInput")
    wvT = nc.dram_tensor("wvT", [D, E], BF, kind="ExternalInput")
    bq = nc.dram_tensor("bq", [E, 1], F32, kind="ExternalInput")
    bk = nc.dram_tensor("bk", [E, 1], F32, kind="ExternalInput")
    bv = nc.dram_tensor("bv", [E, 1], F32, kind="ExternalInput")
    wo2 = nc.dram_tensor("wo2", [E, D], BF, kind="ExternalInput")
    ones64 = nc.dram_tensor("ones64", [128, HD], BF, kind="ExternalInput")
    ident128 = nc.dram_tensor("ident128", [128, 128], BF, kind="ExternalInput")
    out = nc.dram_tensor("out", [BT, D], BF, kind="ExternalOutput")

    with TileContext(nc) as tc:
        with (
            nc.allow_low_precision(reason="bf16 matmuls are deliberate"),
            tc.tile_pool(name="const", bufs=1) as cpool,
            tc.tile_pool(name="pers", bufs=1) as pers,
            tc.tile_pool(name="work", bufs=2) as work,
            tc.tile_pool(name="psum", bufs=2, space="PSUM") as psum,
        ):
            # ---- constants -------------------------------------------------
            wq_sb = cpool.tile([128, D // 128, E], BF, name="wq_sb")
            wk_sb = cpool.tile([128, D // 128, E], BF, name="wk_sb")
            wv_sb = cpool.tile([128, D // 128, E], BF, name="wv_sb")
            nc.sync.dma_start(wq_sb, wqT.rearrange("(n p) m -> p n m", p=128))
            nc.sync.dma_start(wk_sb, wkT.rearrange("(n p) m -> p n m", p=128))
            nc.sync.dma_start(wv_sb, wvT.rearrange("(n p) m -> p n m", p=128))
            wo_sb = cpool.tile([E, D], BF, name="wo_sb")
            nc.sync.dma_start(wo_sb, wo2[:, :])
            bq_sb = cpool.tile([E, 1], F32, name="bq_sb")
            bk_sb = cpool.tile([E, 1], F32, name="bk_sb")
            bv_sb = cpool.tile([E, 1], F32, name="bv_sb")
            nc.sync.dma_start(bq_sb, bq[:, :])
            nc.sync.dma_start(bk_sb, bk[:, :])
            nc.sync.dma_start(bv_sb, bv[:, :])
            ident = cpool.tile([128, 128], BF, name="ident")
            nc.sync.dma_start(ident, ident128[:, :])
            ones_sb = cpool.tile([128, HD], BF, name="ones_sb")
            nc.sync.dma_start(ones_sb, ones64[:, :])

            # ---- persistent activations -----------------------------------
            qT = pers.tile([E, BT], BF, name="qT")
            kT = pers.tile([E, BT], BF, name="kT")
            v = pers.tile([128, NKT, HPC, HD + 1], BF, name="v")
            ctxT = pers.tile([128, BT], BF, name="ctxT")
            nc.sync.dma_start(v[:, :, :, HD], ones64[:, : NKT * HPC])

            # ---- phase A: QKV projections ---------------------------------
            for t in range(NTCH):
                cols = bass.ts(t, TCH)
                xt = work.tile([128, D // 128, TCH], BF, name="xt", tag="xt", bufs=2)
                nc.sync.dma_start(
                    xt, xT[:, cols].rearrange("(n p) m -> p n m", p=128)
                )
                for w_sb, b_sb, dst in (
                    (wq_sb, bq_sb, qT),
                    (wk_sb, bk_sb, kT),
                    (wv_sb, bv_sb, None),
                ):
                    ps = psum.tile([128, TCH], F32, name="ps_mm", tag="mm", bufs=2)
                    for d in range(D // 128):
                        nc.tensor.matmul(
                            ps,
                            lhsT=w_sb[:, d, :],
                            rhs=xt[:, d, :],
                            start=(d == 0),
                            stop=(d == D // 128 - 1),
                        )
                    if dst is not None:
                        nc.scalar.activation(
                            dst[:, cols], ps, AF.Identity, bias=b_sb, scale=1.0
                        )
                    else:
                        vt = work.tile([128, TCH], BF, name="vt", tag="vt", bufs=2)
                        nc.scalar.activation(vt, ps, AF.Identity, bias=b_sb, scale=1.0)
                        # transpose v back to [token, dim] layout, 128 at a time
                        for i in range(TCH // 128):
                            kt_idx = t * (TCH // 128) + i
                            tp = psum.tile(
                                [128, 128], BF, name="tp", tag="s", bufs=2
                            )
                            nc.tensor.transpose(tp, vt[:, bass.ts(i, 128)], ident)
                            for h in range(HPC):
                                nc.vector.tensor_copy(
                                    v[:, kt_idx, h, 0:HD], tp[:, bass.ts(h, HD)]
                                )

            # ---- phase B: attention (flash-style, per (batch, qchunk)) -----
            # Per k-tile, both heads' S^T matmuls are row-tiled (T0/T8) so
            # they run concurrently on the PE; both land in one [128, 1024]
            # PSUM tile so a single Exp serves both heads. ctx matmuls are
            # software-pipelined one 2-k-tile block behind the S matmuls.
            BLK = 2
            NBLK = KTB // BLK
            for b in range(B):
                for qc in range(T // TCH):
                    q0 = b * T + qc * TCH
                    cps = []
                    for h in range(HPC):
                        cp = psum.tile(
                            [HD + 1, TCH], F32, name=f"cp{h}", tag=f"ctx{h}", bufs=1
                        )
                        cps.append(cp)
                    pts = {}
                    for blk in range(NBLK + 1):
                        if blk < NBLK:
                            for kt in range(blk * BLK, (blk + 1) * BLK):
                                k0 = b * T + kt * 128
                                sp = psum.tile(
                                    [128, HPC * TCH], F32, name="sp", tag="s", bufs=2
                                )
                                for h in range(HPC):
                                    he = bass.ts(h, HD)
                                    nc.tensor.matmul(
                                        sp[:, bass.ts(h, TCH)],
                                        lhsT=kT[he, k0 : k0 + 128],
                                        rhs=qT[he, q0 : q0 + TCH],
                                        start=True,
                                        stop=True,
                                    )
                                pt = work.tile(
                                    [128, HPC * TCH], BF, name="pt", tag="pt",
                                    bufs=8,
                                )
                                nc.scalar.activation(pt, sp, AF.Exp, scale=1.0 / 8.0)
                                pts[kt] = pt
                        if blk > 0:
                            for kt in range((blk - 1) * BLK, blk * BLK):
                                for h in range(HPC):
                                    nc.tensor.matmul(
                                        cps[h],
                                        lhsT=v[:, b * KTB + kt, h, :],
                                        rhs=pts[kt][:, bass.ts(h, TCH)],
                                        start=(kt == 0),
                                        stop=(kt == KTB - 1),
                                        skip_group_check=True,
                                    )
                    # normalize: ctxT_h = cp_h[0:64] * (1/cp_h[64]) broadcast.
                    # cp -> sbuf (bf16 cast), reciprocal on the den row, PE
                    # broadcasts it to 64 partitions (K=1 ones outer
                    # product), DVE multiplies.
                    for h in range(HPC):
                        cs = work.tile(
                            [HD + 1, TCH], BF, name="cs", tag="cs", bufs=2
                        )
                        nc.vector.tensor_copy(cs, cps[h])
                        nc.vector.reciprocal(
                            cs[HD : HD + 1, :], cs[HD : HD + 1, :]
                        )
                        rb = psum.tile([HD, TCH], F32, name="rb", tag="mm", bufs=2)
                        nc.tensor.matmul(
                            rb,
                            lhsT=ones_sb[HD : HD + 1, :],
                            rhs=cs[HD : HD + 1, :],
                            start=True,
                            stop=True,
                        )
                        if h == 0:
                            nc.vector.tensor_tensor(
                                ctxT[0:HD, q0 : q0 + TCH],
                                cs[0:HD, :],
                                rb,
                                op=mybir.AluOpType.mult,
                            )
                        else:
                            # h1 lives on partitions 64-127 of ctxT; engines
                            # can't shift partitions, so stage and DMA.
                            ctxs = work.tile(
                                [HD, TCH], BF, name="ctxs", tag="ctxs", bufs=2
                            )
                            nc.vector.tensor_tensor(
                                ctxs,
                                cs[0:HD, :],
                                rb,
                                op=mybir.AluOpType.mult,
                            )
                            nc.sync.dma_start(
                                ctxT[HD:128, q0 : q0 + TCH], ctxs
                            )

            # ---- phase C: output projection (partial over this core's slice)
            # ctxT stacks both heads on partitions -> one K=128 matmul per
            # (token tile, out chunk). po -> ob copies alternate DVE/ACT.
            for tt in range(NKT):
                trows = bass.ts(tt, 128)
                for nch in range(D // TCH):
                    po = psum.tile([128, TCH], F32, name="po", tag="mm", bufs=2)
                    nc.tensor.matmul(
                        po,
                        lhsT=ctxT[:, trows],
                        rhs=wo_sb[:, bass.ts(nch, TCH)],
                        start=True,
                        stop=True,
                    )
                    ob = work.tile([128, TCH], BF, name="ob", tag="ob", bufs=4)
                    if (tt * 2 + nch) % 2 == 0:
                        nc.vector.tensor_copy(ob, po)
                    else:
                        nc.scalar.activation(ob, po, AF.Copy)
                    nc.sync.dma_start(out[trows, bass.ts(nch, TCH)], ob)

    _split_matmul_waits(nc)
    return nc


def _split_matmul_waits(nc):
    """This walrus allows only one sync wait per engine instruction (and none
    on fp32/f32r InstMatmult, whose embedded S3_LW carries the wait slot).
    Move excess waits onto InstEventSemaphore instructions (capacity 2)
    inserted just before the owner in the same engine stream — sequencer
    dispatch is in-order, so semantics are unchanged."""
    ctr = 0
    for f in nc.m.functions:
        for blk in f.blocks:
            out = []
            for inst in blk.instructions:
                si = inst.sync_info
                if (
                    si is not None
                    and not isinstance(inst, mybir.InstEventSemaphore)
                    and len(si.on_wait) > 1
                ):
                    waits = list(si.on_wait)
                    keep = [waits.pop(0)]
                    for i in range(0, len(waits), 2):
                        ev = mybir.InstEventSemaphore(name=f"I-exwait-{ctr}")
                        ctr += 1
                        ev.engine = inst.engine
                        ev.sync_info = mybir.SyncInfo(
                            on_wait=waits[i : i + 2], on_update=[]
                        )
                        nc.register_instruction(ev)
                        out.append(ev)
                    si.on_wait = keep
                out.append(inst)
            blk.instructions[:] = out


_CACHE = {}


def _get_nc():
    if "nc" not in _CACHE:
        _CACHE["nc"] = build_nc()
    return _CACHE["nc"]


def make_in_maps(x, w_qkv, b_qkv, w_out):
    bf = ml_dtypes.bfloat16
    x = np.ascontiguousarray(np.asarray(x, np.float32)).reshape(BT, D)
    w_qkv = np.asarray(w_qkv, np.float32)
    b_qkv = np.asarray(b_qkv, np.float32)
    w_out = np.asarray(w_out, np.float32)

    xT = np.ascontiguousarray(x.T).astype(bf)  # [D, BT]
    wq, wk, wv = w_qkv[0:D], w_qkv[D : 2 * D], w_qkv[2 * D : 3 * D]
    bqs, bks, bvs = b_qkv[0:D], b_qkv[D : 2 * D], b_qkv[2 * D : 3 * D]
    ones = np.ones((128, HD), bf)
    ident = np.eye(128, dtype=np.float32).astype(bf)

    in_maps = []
    for c in range(NCORES):
        rs = slice(E * c, E * (c + 1))
        # wo2[r, o] = w_out[o, E*c + r]  (r = h*64+hd packs both heads on K)
        wo_c = np.ascontiguousarray(w_out[:, rs].T).astype(bf)
        in_maps.append(
            {
                "xT": xT,
                "wqT": np.ascontiguousarray(wq[rs].T).astype(bf),
                "wkT": np.ascontiguousarray(wk[rs].T).astype(bf),
                "wvT": np.ascontiguousarray(wv[rs].T).astype(bf),
                "bq": np.ascontiguousarray(bqs[rs])[:, None],
                "bk": np.ascontiguousarray(bks[rs])[:, None],
                "bv": np.ascontiguousarray(bvs[rs])[:, None],
                "wo2": wo_c,
                "ones64": ones,
                "ident128": ident,
            }
        )
    return in_maps


def _combine(results, b_out):
    acc = results[0]["out"].astype(np.float32)
    for r in results[1:]:
        acc += r["out"].astype(np.float32)
    acc += np.asarray(b_out, np.float32)[None, :]
    return acc.reshape(B, T, D)


def kernel(x, w_qkv, b_qkv, w_out, b_out):
    in_maps = make_in_maps(x, w_qkv, b_qkv, w_out)
    res = run_bass_kernel_spmd(_get_nc(), in_maps, core_ids=list(range(NCORES)))
    return _combine(res.results, b_out)


def kernel_traced(x, w_qkv, b_qkv, w_out, b_out):
    """Like kernel() but profiles the run; returns (output, exec_time_ns)."""
    in_maps = make_in_maps(x, w_qkv, b_qkv, w_out)
    res = run_bass_kernel_spmd(
        _get_nc(), in_maps, core_ids=list(range(NCORES)), trace=True
    )
    return _combine(res.results, b_out), res.exec_time_ns


# revision 8
# speedup vs baseline: 1.5107x; 1.2111x over previous
# Multi-head attention (B=2, T=2048, D=1024, H=16) on 8 TRN2 NeuronCores.
#
# Sharding: tensor-parallel over heads. Each core owns 2 heads (a 128-wide
# slice of the hidden dim): it computes its q/k/v projection slice, full
# attention for its 4 (batch, head) pairs, and a partial output projection
# over its slice of the contraction. The 8 partial outputs are summed on the
# host (the TP all-reduce, done as part of unsharding), plus the output bias.
#
# All matmul operands are bf16 (PSUM accumulation stays fp32): rel tolerance
# is 2e-2 and bf16 keeps us ~2.5e-3, while halving DMA/SBUF traffic and
# letting weight loads overlap matmul streaming.
#
# Layouts (per core):
#   xT   [D=1024, B*T=4096]   x transposed so the contraction dim is on SBUF
#                             partitions for the projection matmuls.
#   qT/kT [128, 4096]         head-dim on partitions (2 heads stacked), token
#                             on free axis -> S^T tiles come out of the PE
#                             directly with softmax's reduction on the free
#                             axis of P^T's consumer.
#   v    [128tok, 32kt, 2h, 65]  natural [token, dim] layout per k-tile with a
#                             ones column appended: the ctx matmul then yields
#                             the softmax denominator for free in row 64.
#   ctxT [128, 4096]          both heads' normalized context stacked on
#                             partitions (h0 rows 0-63, h1 rows 64-127) so the
#                             output projection contracts K=128 in one matmul
#                             per tile.
import sys
import types

import numpy as np


def _install_ntff_hook_shim():
    """This image's `antenv` lacks `axon_hooks`, which bass_utils imports
    unconditionally when tracing is requested (e.g. BASS_TRACE=1). Provide
    the module and register the ctypes NTFF hook the way trn_boot would."""
    try:
        import antenv.axon_hooks  # noqa: F401

        return
    except ImportError:
        pass
    try:
        import antenv
    except ImportError:
        return
    mod = types.ModuleType("antenv.axon_hooks")
    _h = [None]
    mod.set_axon_ntff_profile_hook = lambda h: _h.__setitem__(0, h)
    mod.get_axon_ntff_profile_hook = lambda: _h[0]
    sys.modules["antenv.axon_hooks"] = mod
    antenv.axon_hooks = mod
    try:
        from trn_agent_boot.trn_boot import _ntff_profile_via_ctypes

        mod.set_axon_ntff_profile_hook(
            _ntff_profile_via_ctypes("/opt/axon/libaxon_pjrt.so")
        )
    except Exception:
        pass


_install_ntff_hook_shim()

import ml_dtypes

import concourse.bass as bass
import concourse.mybir as mybir
from concourse.bass_utils import run_bass_kernel_spmd
from concourse.tile import TileContext

B, T, D, H = 2, 2048, 1024, 16
HD = D // H          # 64
NCORES = 8
BT = B * T           # 4096
E = D // NCORES      # 128 = per-core slice of hidden dim (2 heads)
HPC = E // HD        # 2 heads per core

F32 = mybir.dt.float32
BF = mybir.dt.bfloat16
AF = mybir.ActivationFunctionType

TCH = 512            # token chunk for projections / q chunks
NTCH = BT // TCH     # 8
NKT = BT // 128      # 32 token tiles of 128
KTB = T // 128       # 16 k-tiles per batch


def build_nc():
    nc = bass.Bass()

    xT = nc.dram_tensor("xT", [D, BT], BF, kind="ExternalInput")
    wqT = nc.dram_tensor("wqT", [D, E], BF, kind="ExternalInput")
    wkT = nc.dram_tensor("wkT", [D, E], BF, kind="ExternalInput")
    wvT = nc.dram_tensor("wvT", [D, E], BF, kind="ExternalInput")
    bq = nc.dram_tensor("bq", [E, 1], F32, kind="ExternalInput")
    bk = nc.dram_tensor("bk", [E, 1], F32, kind="ExternalInput")
    bv = nc.dram_tensor("bv", [E, 1], F32, kind="ExternalInput")
    wo2 = nc.dram_tensor("wo2", [E, D], BF, kind="ExternalInput")
    ones64 = nc.dram_tensor("ones64", [128, HD], BF, kind="ExternalInput")
    ident128 = nc.dram_tensor("ident128", [128, 128], BF, kind="ExternalInput")
    out = nc.dram_tensor("out", [BT, D], BF, kind="ExternalOutput")

    with TileContext(nc) as tc:
        with (
            nc.allow_low_precision(reason="bf16 matmuls are deliberate"),
            tc.tile_pool(name="const", bufs=1) as cpool,
            tc.tile_pool(name="pers", bufs=1) as pers,
            tc.tile_pool(name="work", bufs=2) as work,
            tc.tile_pool(name="psum", bufs=2, space="PSUM") as psum,
        ):
            # ---- constants -------------------------------------------------
            wq_sb = cpool.tile([128, D // 128, E], BF, name="wq_sb")
            wk_sb = cpool.tile([128, D // 128, E], BF, name="wk_sb")
            wv_sb = cpool.tile([128, D // 128, E], BF, name="wv_sb")
            nc.sync.dma_start(wq_sb, wqT.rearrange("(n p) m -> p n m", p=128))
            nc.sync.dma_start(wk_sb, wkT.rearrange("(n p) m -> p n m", p=128))
            nc.sync.dma_start(wv_sb, wvT.rearrange("(n p) m -> p n m", p=128))
            wo_sb = cpool.tile([E, D], BF, name="wo_sb")
            nc.sync.dma_start(wo_sb, wo2[:, :])
            bq_sb = cpool.tile([E, 1], F32, name="bq_sb")
            bk_sb = cpool.tile([E, 1], F32, name="bk_sb")
            bv_sb = cpool.tile([E, 1], F32, name="bv_sb")
            nc.sync.dma_start(bq_sb, bq[:, :])
            nc.sync.dma_start(bk_sb, bk[:, :])
            nc.sync.dma_start(bv_sb, bv[:, :])
            ident = cpool.tile([128, 128], BF, name="ident")
            nc.sync.dma_start(ident, ident128[:, :])
            ones_sb = cpool.tile([128, HD], BF, name="ones_sb")
            nc.sync.dma_start(ones_sb, ones64[:, :])

            # ---- persistent activations -----------------------------------
            qT = pers.tile([E, BT], BF, name="qT")
            kT = pers.tile([E, BT], BF, name="kT")
            v = pers.tile([128, NKT, HPC, HD + 1], BF, name="v")
            ctxT = pers.tile([128, BT], BF, name="ctxT")
            nc.sync.dma_start(v[:, :, :, HD], ones64[:, : NKT * HPC])

            # ---- phase A: QKV projections ---------------------------------
            for t in range(NTCH):
                cols = bass.ts(t, TCH)
                xt = work.tile([128, D // 128, TCH], BF, name="xt", tag="xt", bufs=2)
                nc.sync.dma_start(
                    xt, xT[:, cols].rearrange("(n p) m -> p n m", p=128)
                )
                for w_sb, b_sb, dst in (
                    (wq_sb, bq_sb, qT),
                    (wk_sb, bk_sb, kT),
                    (wv_sb, bv_sb, None),
                ):
                    ps = psum.tile([128, TCH], F32, name="ps_mm", tag="mm", bufs=2)
                    for d in range(D // 128):
                        nc.tensor.matmul(
                            ps,
                            lhsT=w_sb[:, d, :],
                            rhs=xt[:, d, :],
                            start=(d == 0),
                            stop=(d == D // 128 - 1),
                        )
                    if dst is not None:
                        nc.scalar.activation(
                            dst[:, cols], ps, AF.Identity, bias=b_sb, scale=1.0
                        )
                    else:
                        vt = work.tile([128, TCH], BF, name="vt", tag="vt", bufs=2)
                        nc.scalar.activation(vt, ps, AF.Identity, bias=b_sb, scale=1.0)
                        # transpose v back to [token, dim] layout, 128 at a time
                        for i in range(TCH // 128):
                            kt_idx = t * (TCH // 128) + i
                            tp = psum.tile(
                                [128, 128], BF, name="tp", tag="s", bufs=2
                            )
                            nc.tensor.transpose(tp, vt[:, bass.ts(i, 128)], ident)
                            for h in range(HPC):
                                nc.vector.tensor_copy(
                                    v[:, kt_idx, h, 0:HD], tp[:, bass.ts(h, HD)]
                                )

            # ---- phases B+C: attention + output projection, pipelined ------
            # Per k-tile, both heads' S^T matmuls are row-tiled (T0/T8) so
            # they run concurrently on the PE; both land in one [128, 1024]
            # PSUM tile. Exp runs on the scalar engine except every 4th
            # k-tile, which uses a Schraudolph bit-trick exp on the DVE
            # (bits16(e^x) ~= round(x*128/ln2 + 16233) viewed as bf16).
            # ctx matmuls are pipelined one 2-k-tile block behind.
            #
            # Softmax normalization is deferred: per (h, qc) the ctx psum is
            # staged to SBUF (cs, bf16) and its denominator row is DMA'd into
            # one den_all row; per *batch* a single reciprocal serves all 8
            # rows, and the scale-multiplies + output projection of batch b
            # run interleaved with batch b+1's attention so the PE never
            # idles long enough to re-throttle.
            EXP_DVE_EVERY = 4       # every 4th k-tile's exp on DVE
            EXP_A = 128.0 / float(np.log(2.0)) / 8.0   # folds the 1/8 scale
            EXP_B = 16233.0
            BLK = 2
            NBLK = KTB // BLK
            NQC = T // TCH          # q chunks per batch
            den_alls = {}
            css = {}

            def attn_qchunk(b, qc):
                q0 = b * T + qc * TCH
                cps = []
                for h in range(HPC):
                    cp = psum.tile(
                        [HD + 1, TCH], F32, name=f"cp{h}", tag=f"ctx{h}", bufs=1
                    )
                    cps.append(cp)
                pts = {}
                for blk in range(NBLK + 1):
                    if blk < NBLK:
                        for kt in range(blk * BLK, (blk + 1) * BLK):
                            k0 = b * T + kt * 128
                            sp = psum.tile(
                                [128, HPC * TCH], F32, name="sp", tag="s", bufs=2
                            )
                            for h in range(HPC):
                                he = bass.ts(h, HD)
                                nc.tensor.matmul(
                                    sp[:, bass.ts(h, TCH)],
                                    lhsT=kT[he, k0 : k0 + 128],
                                    rhs=qT[he, q0 : q0 + TCH],
                                    start=True,
                                    stop=True,
                                )
                            pt = work.tile(
                                [128, HPC * TCH], BF, name="pt", tag="pt",
                                bufs=8,
                            )
                            if kt % EXP_DVE_EVERY == EXP_DVE_EVERY - 1:
                                nc.vector.tensor_scalar(
                                    pt.bitcast(mybir.dt.int16), sp,
                                    EXP_A, EXP_B,
                                    op0=mybir.AluOpType.mult,
                                    op1=mybir.AluOpType.add,
                                )
                            else:
                                nc.scalar.activation(
                                    pt, sp, AF.Exp, scale=1.0 / 8.0
                                )
                            pts[kt] = pt
                    if blk > 0:
                        for kt in range((blk - 1) * BLK, blk * BLK):
                            for h in range(HPC):
                                nc.tensor.matmul(
                                    cps[h],
                                    lhsT=v[:, b * KTB + kt, h, :],
                                    rhs=pts[kt][:, bass.ts(h, TCH)],
                                    start=(kt == 0),
                                    stop=(kt == KTB - 1),
                                    skip_group_check=True,
                                )
                # stage ctx+den to SBUF (frees psum); den rows collect into
                # den_all[b] via partition-shifting DMA for one batched recip.
                for h in range(HPC):
                    cs = work.tile(
                        [HD + 1, TCH], BF, name="cs", tag="cs", bufs=2 * NQC * HPC
                    )
                    nc.vector.tensor_copy(cs, cps[h])
                    nc.sync.dma_start(
                        den_alls[b][qc * HPC + h : qc * HPC + h + 1, :],
                        cs[HD : HD + 1, :],
                    )
                    css[(b, qc, h)] = cs

            def norm_recip(b):
                nc.vector.reciprocal(den_alls[b], den_alls[b])
                # LDWEIGHTS needs 32-aligned partitions: shift each recip row
                # back to partition 0 so the broadcast matmuls can read it.
                for i in range(NQC * HPC):
                    nc.sync.dma_start(
                        rdens[b][0:1, i, :], den_alls[b][i : i + 1, :]
                    )

            def norm_mult(b, qc, h):
                q0 = b * T + qc * TCH
                i = qc * HPC + h
                cs = css.pop((b, qc, h))
                rb = psum.tile([HD, TCH], F32, name="rb", tag="mm", bufs=2)
                nc.tensor.matmul(
                    rb,
                    lhsT=ones_sb[0:1, :],
                    rhs=rdens[b][0:1, i, :],
                    start=True,
                    stop=True,
                )
                if h == 0:
                    nc.vector.tensor_tensor(
                        ctxT[0:HD, q0 : q0 + TCH],
                        cs[0:HD, :],
                        rb,
                        op=mybir.AluOpType.mult,
                    )
                else:
                    # h1 lives on partitions 64-127 of ctxT; engines can't
                    # shift partitions, so stage and DMA.
                    ctxs = work.tile(
                        [HD, TCH], BF, name="ctxs", tag="ctxs", bufs=2
                    )
                    nc.vector.tensor_tensor(
                        ctxs,
                        cs[0:HD, :],
                        rb,
                        op=mybir.AluOpType.mult,
                    )
                    nc.sync.dma_start(ctxT[HD:128, q0 : q0 + TCH], ctxs)

            def out_proj_tile(tt, j):
                # ctxT stacks both heads on partitions -> one K=128 matmul
                # per (token tile, out chunk). po -> ob copies alternate
                # ACT/DVE.
                trows = bass.ts(tt, 128)
                for nch in range(D // TCH):
                    po = psum.tile([128, TCH], F32, name="po", tag="mm", bufs=2)
                    nc.tensor.matmul(
                        po,
                        lhsT=ctxT[:, trows],
                        rhs=wo_sb[:, bass.ts(nch, TCH)],
                        start=True,
                        stop=True,
                    )
                    ob = work.tile([128, TCH], BF, name="ob", tag="ob", bufs=4)
                    if (j + nch) % 2 == 0:
                        nc.scalar.activation(ob, po, AF.Copy)
                    else:
                        nc.vector.tensor_copy(ob, po)
                    nc.sync.dma_start(out[trows, bass.ts(nch, TCH)], ob)

            rdens = {}
            for b in range(B):
                den_alls[b] = work.tile(
                    [NQC * HPC, TCH], BF, name=f"den_all{b}", tag=f"den{b}",
                    bufs=1,
                )
                rdens[b] = work.tile(
                    [1, NQC * HPC, TCH], BF, name=f"rden{b}", tag=f"rden{b}",
                    bufs=1,
                )

            # batch 0 attention
            for qc in range(NQC):
                attn_qchunk(0, qc)
            # batch 1 attention, interleaved with batch 0's normalize + proj
            norm_recip(0)
            for qc in range(NQC):
                attn_qchunk(1, qc)
                for h in range(HPC):
                    norm_mult(0, qc, h)
                for tt in range(qc * (KTB // NQC), (qc + 1) * (KTB // NQC)):
                    out_proj_tile(tt, 2 * tt)
            # batch 0's projection tail + batch 1 normalize + projection
            norm_recip(1)
            for qc in range(NQC):
                for h in range(HPC):
                    norm_mult(1, qc, h)
                for tt in range(qc * (KTB // NQC), (qc + 1) * (KTB // NQC)):
                    out_proj_tile(KTB + tt, 2 * tt)

    _split_matmul_waits(nc)
    return nc


def _split_matmul_waits(nc):
    """This walrus allows only one sync wait per engine instruction (and none
    on fp32/f32r InstMatmult, whose embedded S3_LW carries the wait slot).
    Move excess waits onto InstEventSemaphore instructions (capacity 2)
    inserted just before the owner in the same engine stream — sequencer
    dispatch is in-order, so semantics are unchanged."""
    ctr = 0
    for f in nc.m.functions:
        for blk in f.blocks:
            out = []
            for inst in blk.instructions:
                si = inst.sync_info
                if (
                    si is not None
                    and not isinstance(inst, mybir.InstEventSemaphore)
                    and len(si.on_wait) > 1
                ):
                    waits = list(si.on_wait)
                    keep = [waits.pop(0)]
                    for i in range(0, len(waits), 2):
                        ev = mybir.InstEventSemaphore(name=f"I-exwait-{ctr}")
                        ctr += 1
                        ev.engine = inst.engine
                        ev.sync_info = mybir.SyncInfo(
                            on_wait=waits[i : i + 2], on_update=[]
                        )
                        nc.register_instruction(ev)
                        out.append(ev)
                    si.on_wait = keep
                out.append(inst)
            blk.instructions[:] = out


_CACHE = {}


def _get_nc():
    if "nc" not in _CACHE:
        _CACHE["nc"] = build_nc()
    return _CACHE["nc"]


def make_in_maps(x, w_qkv, b_qkv, w_out):
    bf = ml_dtypes.bfloat16
    x = np.ascontiguousarray(np.asarray(x, np.float32)).reshape(BT, D)
    w_qkv = np.asarray(w_qkv, np.float32)
    b_qkv = np.asarray(b_qkv, np.float32)
    w_out = np.asarray(w_out, np.float32)

    xT = np.ascontiguousarray(x.T).astype(bf)  # [D, BT]
    wq, wk, wv = w_qkv[0:D], w_qkv[D : 2 * D], w_qkv[2 * D : 3 * D]
    bqs, bks, bvs = b_qkv[0:D], b_qkv[D : 2 * D], b_qkv[2 * D : 3 * D]
    ones = np.ones((128, HD), bf)
    ident = np.eye(128, dtype=np.float32).astype(bf)

    in_maps = []
    for c in range(NCORES):
        rs = slice(E * c, E * (c + 1))
        # wo2[r, o] = w_out[o, E*c + r]  (r = h*64+hd packs both heads on K)
        wo_c = np.ascontiguousarray(w_out[:, rs].T).astype(bf)
        in_maps.append(
            {
                "xT": xT,
                "wqT": np.ascontiguousarray(wq[rs].T).astype(bf),
                "wkT": np.ascontiguousarray(wk[rs].T).astype(bf),
                "wvT": np.ascontiguousarray(wv[rs].T).astype(bf),
                "bq": np.ascontiguousarray(bqs[rs])[:, None],
                "bk": np.ascontiguousarray(bks[rs])[:, None],
                "bv": np.ascontiguousarray(bvs[rs])[:, None],
                "wo2": wo_c,
                "ones64": ones,
                "ident128": ident,
            }
        )
    return in_maps


def _combine(results, b_out):
    acc = results[0]["out"].astype(np.float32)
    for r in results[1:]:
        acc += r["out"].astype(np.float32)
    acc += np.asarray(b_out, np.float32)[None, :]
    return acc.reshape(B, T, D)


def kernel(x, w_qkv, b_qkv, w_out, b_out):
    in_maps = make_in_maps(x, w_qkv, b_qkv, w_out)
    res = run_bass_kernel_spmd(_get_nc(), in_maps, core_ids=list(range(NCORES)))
    return _combine(res.results, b_out)


def kernel_traced(x, w_qkv, b_qkv, w_out, b_out):
    """Like kernel() but profiles the run; returns (output, exec_time_ns)."""
    in_maps = make_in_maps(x, w_qkv, b_qkv, w_out)
    res = run_bass_kernel_spmd(
        _get_nc(), in_maps, core_ids=list(range(NCORES)), trace=True
    )
    return _combine(res.results, b_out), res.exec_time_ns


# revision 9
# speedup vs baseline: 1.5150x; 1.0028x over previous
# Multi-head attention (B=2, T=2048, D=1024, H=16) on 8 TRN2 NeuronCores.
#
# Sharding: tensor-parallel over heads. Each core owns 2 heads (a 128-wide
# slice of the hidden dim): it computes its q/k/v projection slice, full
# attention for its 4 (batch, head) pairs, and a partial output projection
# over its slice of the contraction. The 8 partial outputs are summed on the
# host (the TP all-reduce, done as part of unsharding), plus the output bias.
#
# All matmul operands are bf16 (PSUM accumulation stays fp32): rel tolerance
# is 2e-2 and bf16 keeps us ~2.5e-3, while halving DMA/SBUF traffic and
# letting weight loads overlap matmul streaming.
#
# Layouts (per core):
#   xT   [D=1024, B*T=4096]   x transposed so the contraction dim is on SBUF
#                             partitions for the projection matmuls.
#   qT/kT [128, 4096]         head-dim on partitions (2 heads stacked), token
#                             on free axis -> S^T tiles come out of the PE
#                             directly with softmax's reduction on the free
#                             axis of P^T's consumer.
#   v    [128tok, 32kt, 2h, 65]  natural [token, dim] layout per k-tile with a
#                             ones column appended: the ctx matmul then yields
#                             the softmax denominator for free in row 64.
#   ctxT [128, 4096]          both heads' normalized context stacked on
#                             partitions (h0 rows 0-63, h1 rows 64-127) so the
#                             output projection contracts K=128 in one matmul
#                             per tile.
import sys
import types

import numpy as np


def _install_ntff_hook_shim():
    """This image's `antenv` lacks `axon_hooks`, which bass_utils imports
    unconditionally when tracing is requested (e.g. BASS_TRACE=1). Provide
    the module and register the ctypes NTFF hook the way trn_boot would."""
    try:
        import antenv.axon_hooks  # noqa: F401

        return
    except ImportError:
        pass
    try:
        import antenv
    except ImportError:
        return
    mod = types.ModuleType("antenv.axon_hooks")
    _h = [None]
    mod.set_axon_ntff_profile_hook = lambda h: _h.__setitem__(0, h)
    mod.get_axon_ntff_profile_hook = lambda: _h[0]
    sys.modules["antenv.axon_hooks"] = mod
    antenv.axon_hooks = mod
    try:
        from trn_agent_boot.trn_boot import _ntff_profile_via_ctypes

        mod.set_axon_ntff_profile_hook(
            _ntff_profile_via_ctypes("/opt/axon/libaxon_pjrt.so")
        )
    except Exception:
        pass


_install_ntff_hook_shim()

import ml_dtypes

import concourse.bass as bass
import concourse.mybir as mybir
from concourse.bass_utils import run_bass_kernel_spmd
from concourse.tile import TileContext

B, T, D, H = 2, 2048, 1024, 16
HD = D // H          # 64
NCORES = 8
BT = B * T           # 4096
E = D // NCORES      # 128 = per-core slice of hidden dim (2 heads)
HPC = E // HD        # 2 heads per core

F32 = mybir.dt.float32
BF = mybir.dt.bfloat16
AF = mybir.ActivationFunctionType

TCH = 512            # token chunk for projections / q chunks
NTCH = BT // TCH     # 8
NKT = BT // 128      # 32 token tiles of 128
KTB = T // 128       # 16 k-tiles per batch


def build_nc():
    nc = bass.Bass()

    xT = nc.dram_tensor("xT", [D, BT], BF, kind="ExternalInput")
    wqT = nc.dram_tensor("wqT", [D, E], BF, kind="ExternalInput")
    wkT = nc.dram_tensor("wkT", [D, E], BF, kind="ExternalInput")
    wvT = nc.dram_tensor("wvT", [D, E], BF, kind="ExternalInput")
    bq = nc.dram_tensor("bq", [E, 1], F32, kind="ExternalInput")
    bk = nc.dram_tensor("bk", [E, 1], F32, kind="ExternalInput")
    bv = nc.dram_tensor("bv", [E, 1], F32, kind="ExternalInput")
    wo2 = nc.dram_tensor("wo2", [E, D], BF, kind="ExternalInput")
    ones64 = nc.dram_tensor("ones64", [128, HD], BF, kind="ExternalInput")
    ident128 = nc.dram_tensor("ident128", [128, 128], BF, kind="ExternalInput")
    out = nc.dram_tensor("out", [BT, D], BF, kind="ExternalOutput")

    with TileContext(nc) as tc:
        with (
            nc.allow_low_precision(reason="bf16 matmuls are deliberate"),
            tc.tile_pool(name="const", bufs=1) as cpool,
            tc.tile_pool(name="pers", bufs=1) as pers,
            tc.tile_pool(name="work", bufs=2) as work,
            tc.tile_pool(name="psum", bufs=2, space="PSUM") as psum,
        ):
            # ---- constants -------------------------------------------------
            wq_sb = cpool.tile([128, D // 128, E], BF, name="wq_sb")
            wk_sb = cpool.tile([128, D // 128, E], BF, name="wk_sb")
            wv_sb = cpool.tile([128, D // 128, E], BF, name="wv_sb")
            nc.sync.dma_start(wq_sb, wqT.rearrange("(n p) m -> p n m", p=128))
            nc.sync.dma_start(wk_sb, wkT.rearrange("(n p) m -> p n m", p=128))
            nc.sync.dma_start(wv_sb, wvT.rearrange("(n p) m -> p n m", p=128))
            wo_sb = cpool.tile([E, D], BF, name="wo_sb")
            nc.sync.dma_start(wo_sb, wo2[:, :])
            bq_sb = cpool.tile([E, 1], F32, name="bq_sb")
            bk_sb = cpool.tile([E, 1], F32, name="bk_sb")
            bv_sb = cpool.tile([E, 1], F32, name="bv_sb")
            nc.sync.dma_start(bq_sb, bq[:, :])
            nc.sync.dma_start(bk_sb, bk[:, :])
            nc.sync.dma_start(bv_sb, bv[:, :])
            ident = cpool.tile([128, 128], BF, name="ident")
            nc.sync.dma_start(ident, ident128[:, :])
            ones_sb = cpool.tile([128, HD], BF, name="ones_sb")
            nc.sync.dma_start(ones_sb, ones64[:, :])

            # ---- persistent activations -----------------------------------
            qT = pers.tile([E, BT], BF, name="qT")
            kT = pers.tile([E, BT], BF, name="kT")
            v = pers.tile([128, NKT, HPC, HD + 1], BF, name="v")
            ctxT = pers.tile([128, BT], BF, name="ctxT")
            nc.sync.dma_start(v[:, :, :, HD], ones64[:, : NKT * HPC])

            # ---- phase A: QKV projections ---------------------------------
            for t in range(NTCH):
                cols = bass.ts(t, TCH)
                xt = work.tile([128, D // 128, TCH], BF, name="xt", tag="xt", bufs=2)
                nc.sync.dma_start(
                    xt, xT[:, cols].rearrange("(n p) m -> p n m", p=128)
                )
                for w_sb, b_sb, dst in (
                    (wq_sb, bq_sb, qT),
                    (wk_sb, bk_sb, kT),
                    (wv_sb, bv_sb, None),
                ):
                    ps = psum.tile([128, TCH], F32, name="ps_mm", tag="mm", bufs=2)
                    for d in range(D // 128):
                        nc.tensor.matmul(
                            ps,
                            lhsT=w_sb[:, d, :],
                            rhs=xt[:, d, :],
                            start=(d == 0),
                            stop=(d == D // 128 - 1),
                        )
                    if dst is not None:
                        nc.scalar.activation(
                            dst[:, cols], ps, AF.Identity, bias=b_sb, scale=1.0
                        )
                    else:
                        vt = work.tile([128, TCH], BF, name="vt", tag="vt", bufs=2)
                        nc.scalar.activation(vt, ps, AF.Identity, bias=b_sb, scale=1.0)
                        # transpose v back to [token, dim] layout, 128 at a time
                        for i in range(TCH // 128):
                            kt_idx = t * (TCH // 128) + i
                            tp = psum.tile(
                                [128, 128], BF, name="tp", tag="s", bufs=2
                            )
                            nc.tensor.transpose(tp, vt[:, bass.ts(i, 128)], ident)
                            for h in range(HPC):
                                nc.vector.tensor_copy(
                                    v[:, kt_idx, h, 0:HD], tp[:, bass.ts(h, HD)]
                                )

            # ---- phases B+C: attention + output projection, pipelined ------
            # Per k-tile, both heads' S^T matmuls are row-tiled (T0/T8) so
            # they run concurrently on the PE; both land in one [128, 1024]
            # PSUM tile. Exp runs on the scalar engine except every 4th
            # k-tile, which uses a Schraudolph bit-trick exp on the DVE
            # (bits16(e^x) ~= round(x*128/ln2 + 16233) viewed as bf16).
            # ctx matmuls are pipelined one 2-k-tile block behind.
            #
            # Softmax normalization is deferred: per (h, qc) the ctx psum is
            # staged to SBUF (cs, bf16) and its denominator row is DMA'd into
            # one den_all row; per *batch* a single reciprocal serves all 8
            # rows, and the scale-multiplies + output projection of batch b
            # run interleaved with batch b+1's attention so the PE never
            # idles long enough to re-throttle.
            EXP_DVE_EVERY = 4       # every 4th k-tile's exp on DVE
            EXP_A = 128.0 / float(np.log(2.0)) / 8.0   # folds the 1/8 scale
            EXP_B = 16249.0  # calibrated for zero mean bias vs exact exp
            BLK = 2
            NBLK = KTB // BLK
            NQC = T // TCH          # q chunks per batch
            den_alls = {}
            css = {}

            def attn_qchunk(b, qc):
                q0 = b * T + qc * TCH
                cps = []
                for h in range(HPC):
                    cp = psum.tile(
                        [HD + 1, TCH], F32, name=f"cp{h}", tag=f"ctx{h}", bufs=1
                    )
                    cps.append(cp)
                pts = {}
                for blk in range(NBLK + 1):
                    if blk < NBLK:
                        for kt in range(blk * BLK, (blk + 1) * BLK):
                            k0 = b * T + kt * 128
                            sp = psum.tile(
                                [128, HPC * TCH], F32, name="sp", tag="s", bufs=2
                            )
                            for h in range(HPC):
                                he = bass.ts(h, HD)
                                nc.tensor.matmul(
                                    sp[:, bass.ts(h, TCH)],
                                    lhsT=kT[he, k0 : k0 + 128],
                                    rhs=qT[he, q0 : q0 + TCH],
                                    start=True,
                                    stop=True,
                                )
                            pt = work.tile(
                                [128, HPC * TCH], BF, name="pt", tag="pt",
                                bufs=8,
                            )
                            if kt % EXP_DVE_EVERY == EXP_DVE_EVERY - 1:
                                nc.vector.tensor_scalar(
                                    pt.bitcast(mybir.dt.int16), sp,
                                    EXP_A, EXP_B,
                                    op0=mybir.AluOpType.mult,
                                    op1=mybir.AluOpType.add,
                                )
                            else:
                                nc.scalar.activation(
                                    pt, sp, AF.Exp, scale=1.0 / 8.0
                                )
                            pts[kt] = pt
                    if blk > 0:
                        for kt in range((blk - 1) * BLK, blk * BLK):
                            for h in range(HPC):
                                nc.tensor.matmul(
                                    cps[h],
                                    lhsT=v[:, b * KTB + kt, h, :],
                                    rhs=pts[kt][:, bass.ts(h, TCH)],
                                    start=(kt == 0),
                                    stop=(kt == KTB - 1),
                                    skip_group_check=True,
                                )
                # stage ctx+den to SBUF (frees psum); den rows collect into
                # den_all[b] via partition-shifting DMA for one batched recip.
                for h in range(HPC):
                    cs = work.tile(
                        [HD + 1, TCH], BF, name="cs", tag="cs", bufs=2 * NQC * HPC
                    )
                    nc.vector.tensor_copy(cs, cps[h])
                    nc.sync.dma_start(
                        den_alls[b][qc * HPC + h : qc * HPC + h + 1, :],
                        cs[HD : HD + 1, :],
                    )
                    css[(b, qc, h)] = cs

            def norm_recip(b):
                nc.vector.reciprocal(den_alls[b], den_alls[b])
                # LDWEIGHTS needs 32-aligned partitions: shift each recip row
                # back to partition 0 so the broadcast matmuls can read it.
                for i in range(NQC * HPC):
                    nc.sync.dma_start(
                        rdens[b][0:1, i, :], den_alls[b][i : i + 1, :]
                    )

            def norm_mult(b, qc, h):
                q0 = b * T + qc * TCH
                i = qc * HPC + h
                cs = css.pop((b, qc, h))
                rb = psum.tile([HD, TCH], F32, name="rb", tag="mm", bufs=2)
                nc.tensor.matmul(
                    rb,
                    lhsT=ones_sb[0:1, :],
                    rhs=rdens[b][0:1, i, :],
                    start=True,
                    stop=True,
                )
                if h == 0:
                    nc.vector.tensor_tensor(
                        ctxT[0:HD, q0 : q0 + TCH],
                        cs[0:HD, :],
                        rb,
                        op=mybir.AluOpType.mult,
                    )
                else:
                    # h1 lives on partitions 64-127 of ctxT; engines can't
                    # shift partitions, so stage and DMA.
                    ctxs = work.tile(
                        [HD, TCH], BF, name="ctxs", tag="ctxs", bufs=2
                    )
                    nc.vector.tensor_tensor(
                        ctxs,
                        cs[0:HD, :],
                        rb,
                        op=mybir.AluOpType.mult,
                    )
                    nc.sync.dma_start(ctxT[HD:128, q0 : q0 + TCH], ctxs)

            def out_proj_tile(tt, j):
                # ctxT stacks both heads on partitions -> one K=128 matmul
                # per (token tile, out chunk). po -> ob copies alternate
                # ACT/DVE.
                trows = bass.ts(tt, 128)
                for nch in range(D // TCH):
                    po = psum.tile([128, TCH], F32, name="po", tag="mm", bufs=2)
                    nc.tensor.matmul(
                        po,
                        lhsT=ctxT[:, trows],
                        rhs=wo_sb[:, bass.ts(nch, TCH)],
                        start=True,
                        stop=True,
                    )
                    ob = work.tile([128, TCH], BF, name="ob", tag="ob", bufs=4)
                    if (j + nch) % 2 == 0:
                        nc.scalar.activation(ob, po, AF.Copy)
                    else:
                        nc.vector.tensor_copy(ob, po)
                    nc.sync.dma_start(out[trows, bass.ts(nch, TCH)], ob)

            rdens = {}
            for b in range(B):
                den_alls[b] = work.tile(
                    [NQC * HPC, TCH], BF, name=f"den_all{b}", tag=f"den{b}",
                    bufs=1,
                )
                rdens[b] = work.tile(
                    [1, NQC * HPC, TCH], BF, name=f"rden{b}", tag=f"rden{b}",
                    bufs=1,
                )

            # batch 0 attention
            for qc in range(NQC):
                attn_qchunk(0, qc)
            # batch 1 attention, interleaved with batch 0's normalize + proj
            norm_recip(0)
            for qc in range(NQC):
                attn_qchunk(1, qc)
                for h in range(HPC):
                    norm_mult(0, qc, h)
                for tt in range(qc * (KTB // NQC), (qc + 1) * (KTB // NQC)):
                    out_proj_tile(tt, 2 * tt)
            # batch 0's projection tail + batch 1 normalize + projection
            norm_recip(1)
            for qc in range(NQC):
                for h in range(HPC):
                    norm_mult(1, qc, h)
                for tt in range(qc * (KTB // NQC), (qc + 1) * (KTB // NQC)):
                    out_proj_tile(KTB + tt, 2 * tt)

    _split_matmul_waits(nc)
    return nc


def _split_matmul_waits(nc):
    """This walrus allows only one sync wait per engine instruction (and none
    on fp32/f32r InstMatmult, whose embedded S3_LW carries the wait slot).
    Move excess waits onto InstEventSemaphore instructions (capacity 2)
    inserted just before the owner in the same engine stream — sequencer
    dispatch is in-order, so semantics are unchanged."""
    ctr = 0
    for f in nc.m.functions:
        for blk in f.blocks:
            out = []
            for inst in blk.instructions:
                si = inst.sync_info
                if (
                    si is not None
                    and not isinstance(inst, mybir.InstEventSemaphore)
                    and len(si.on_wait) > 1
                ):
                    waits = list(si.on_wait)
                    keep = [waits.pop(0)]
                    for i in range(0, len(waits), 2):
                        ev = mybir.InstEventSemaphore(name=f"I-exwait-{ctr}")
                        ctr += 1
                        ev.engine = inst.engine
                        ev.sync_info = mybir.SyncInfo(
                            on_wait=waits[i : i + 2], on_update=[]
                        )
                        nc.register_instruction(ev)
                        out.append(ev)
                    si.on_wait = keep
                out.append(inst)
            blk.instructions[:] = out


_CACHE = {}


def _get_nc():
    if "nc" not in _CACHE:
        _CACHE["nc"] = build_nc()
    return _CACHE["nc"]


def make_in_maps(x, w_qkv, b_qkv, w_out):
    bf = ml_dtypes.bfloat16
    x = np.ascontiguousarray(np.asarray(x, np.float32)).reshape(BT, D)
    w_qkv = np.asarray(w_qkv, np.float32)
    b_qkv = np.asarray(b_qkv, np.float32)
    w_out = np.asarray(w_out, np.float32)

    xT = np.ascontiguousarray(x.T).astype(bf)  # [D, BT]
    wq, wk, wv = w_qkv[0:D], w_qkv[D : 2 * D], w_qkv[2 * D : 3 * D]
    bqs, bks, bvs = b_qkv[0:D], b_qkv[D : 2 * D], b_qkv[2 * D : 3 * D]
    ones = np.ones((128, HD), bf)
    ident = np.eye(128, dtype=np.float32).astype(bf)

    in_maps = []
    for c in range(NCORES):
        rs = slice(E * c, E * (c + 1))
        # wo2[r, o] = w_out[o, E*c + r]  (r = h*64+hd packs both heads on K)
        wo_c = np.ascontiguousarray(w_out[:, rs].T).astype(bf)
        in_maps.append(
            {
                "xT": xT,
                "wqT": np.ascontiguousarray(wq[rs].T).astype(bf),
                "wkT": np.ascontiguousarray(wk[rs].T).astype(bf),
                "wvT": np.ascontiguousarray(wv[rs].T).astype(bf),
                "bq": np.ascontiguousarray(bqs[rs])[:, None],
                "bk": np.ascontiguousarray(bks[rs])[:, None],
                "bv": np.ascontiguousarray(bvs[rs])[:, None],
                "wo2": wo_c,
                "ones64": ones,
                "ident128": ident,
            }
        )
    return in_maps


def _combine(results, b_out):
    acc = results[0]["out"].astype(np.float32)
    for r in results[1:]:
        acc += r["out"].astype(np.float32)
    acc += np.asarray(b_out, np.float32)[None, :]
    return acc.reshape(B, T, D)


def kernel(x, w_qkv, b_qkv, w_out, b_out):
    in_maps = make_in_maps(x, w_qkv, b_qkv, w_out)
    res = run_bass_kernel_spmd(_get_nc(), in_maps, core_ids=list(range(NCORES)))
    return _combine(res.results, b_out)


def kernel_traced(x, w_qkv, b_qkv, w_out, b_out):
    """Like kernel() but profiles the run; returns (output, exec_time_ns)."""
    in_maps = make_in_maps(x, w_qkv, b_qkv, w_out)
    res = run_bass_kernel_spmd(
        _get_nc(), in_maps, core_ids=list(range(NCORES)), trace=True
    )
    return _combine(res.results, b_out), res.exec_time_ns


# revision 12
# speedup vs baseline: 1.5326x; 1.0116x over previous
# Multi-head attention (B=2, T=2048, D=1024, H=16) on 8 TRN2 NeuronCores.
#
# Sharding: tensor-parallel over heads. Each core owns 2 heads (a 128-wide
# slice of the hidden dim): it computes its q/k/v projection slice, full
# attention for its 4 (batch, head) pairs, and a partial output projection
# over its slice of the contraction. The 8 partial outputs are summed on the
# host (the TP all-reduce, done as part of unsharding), plus the output bias.
#
# All matmul operands are bf16 (PSUM accumulation stays fp32): rel tolerance
# is 2e-2 and bf16 keeps us ~2.5e-3, while halving DMA/SBUF traffic and
# letting weight loads overlap matmul streaming.
#
# Layouts (per core):
#   xT   [D=1024, B*T=4096]   x transposed so the contraction dim is on SBUF
#                             partitions for the projection matmuls.
#   qT/kT [128, 4096]         head-dim on partitions (2 heads stacked), token
#                             on free axis -> S^T tiles come out of the PE
#                             directly with softmax's reduction on the free
#                             axis of P^T's consumer.
#   v    [128tok, 32kt, 2h, 65]  natural [token, dim] layout per k-tile with a
#                             ones column appended: the ctx matmul then yields
#                             the softmax denominator for free in row 64.
#   ctxT [128, 4096]          both heads' normalized context stacked on
#                             partitions (h0 rows 0-63, h1 rows 64-127) so the
#                             output projection contracts K=128 in one matmul
#                             per tile.
import sys
import types

import numpy as np


def _install_ntff_hook_shim():
    """This image's `antenv` lacks `axon_hooks`, which bass_utils imports
    unconditionally when tracing is requested (e.g. BASS_TRACE=1). Provide
    the module and register the ctypes NTFF hook the way trn_boot would."""
    try:
        import antenv.axon_hooks  # noqa: F401

        return
    except ImportError:
        pass
    try:
        import antenv
    except ImportError:
        return
    mod = types.ModuleType("antenv.axon_hooks")
    _h = [None]
    mod.set_axon_ntff_profile_hook = lambda h: _h.__setitem__(0, h)
    mod.get_axon_ntff_profile_hook = lambda: _h[0]
    sys.modules["antenv.axon_hooks"] = mod
    antenv.axon_hooks = mod
    try:
        from trn_agent_boot.trn_boot import _ntff_profile_via_ctypes

        mod.set_axon_ntff_profile_hook(
            _ntff_profile_via_ctypes("/opt/axon/libaxon_pjrt.so")
        )
    except Exception:
        pass


_install_ntff_hook_shim()

import ml_dtypes

import concourse.bass as bass
import concourse.mybir as mybir
from concourse.bass_utils import run_bass_kernel_spmd
from concourse.tile import TileContext

B, T, D, H = 2, 2048, 1024, 16
HD = D // H          # 64
NCORES = 8
BT = B * T           # 4096
E = D // NCORES      # 128 = per-core slice of hidden dim (2 heads)
HPC = E // HD        # 2 heads per core

F32 = mybir.dt.float32
BF = mybir.dt.bfloat16
AF = mybir.ActivationFunctionType

TCH = 512            # token chunk for projections / q chunks
NTCH = BT // TCH     # 8
NKT = BT // 128      # 32 token tiles of 128
KTB = T // 128       # 16 k-tiles per batch


def build_nc():
    nc = bass.Bass()

    xT = nc.dram_tensor("xT", [D, BT], BF, kind="ExternalInput")
    wqT = nc.dram_tensor("wqT", [D, E], BF, kind="ExternalInput")
    wkT = nc.dram_tensor("wkT", [D, E], BF, kind="ExternalInput")
    wvT = nc.dram_tensor("wvT", [D, E], BF, kind="ExternalInput")
    bq = nc.dram_tensor("bq", [E, 1], F32, kind="ExternalInput")
    bk = nc.dram_tensor("bk", [E, 1], F32, kind="ExternalInput")
    bv = nc.dram_tensor("bv", [E, 1], F32, kind="ExternalInput")
    wo2 = nc.dram_tensor("wo2", [E, D], BF, kind="ExternalInput")
    ones64 = nc.dram_tensor("ones64", [128, HD], BF, kind="ExternalInput")
    ident128 = nc.dram_tensor("ident128", [128, 128], BF, kind="ExternalInput")
    out = nc.dram_tensor("out", [BT, D], BF, kind="ExternalOutput")

    with TileContext(nc) as tc:
        with (
            nc.allow_low_precision(reason="bf16 matmuls are deliberate"),
            tc.tile_pool(name="const", bufs=1) as cpool,
            tc.tile_pool(name="pers", bufs=1) as pers,
            tc.tile_pool(name="work", bufs=2) as work,
            tc.tile_pool(name="psum", bufs=2, space="PSUM") as psum,
        ):
            # ---- constants (x chunk 0 is DMA'd first in phase A; wo_sb
            # isn't needed until the first out-projection, so it loads last)
            wq_sb = cpool.tile([128, D // 128, E], BF, name="wq_sb")
            wk_sb = cpool.tile([128, D // 128, E], BF, name="wk_sb")
            wv_sb = cpool.tile([128, D // 128, E], BF, name="wv_sb")
            nc.sync.dma_start(wq_sb, wqT.rearrange("(n p) m -> p n m", p=128))
            nc.sync.dma_start(wk_sb, wkT.rearrange("(n p) m -> p n m", p=128))
            nc.sync.dma_start(wv_sb, wvT.rearrange("(n p) m -> p n m", p=128))
            bq_sb = cpool.tile([E, 1], F32, name="bq_sb")
            bk_sb = cpool.tile([E, 1], F32, name="bk_sb")
            bv_sb = cpool.tile([E, 1], F32, name="bv_sb")
            nc.sync.dma_start(bq_sb, bq[:, :])
            nc.sync.dma_start(bk_sb, bk[:, :])
            nc.sync.dma_start(bv_sb, bv[:, :])
            ident = cpool.tile([128, 128], BF, name="ident")
            nc.sync.dma_start(ident, ident128[:, :])
            ones_sb = cpool.tile([128, HD], BF, name="ones_sb")
            nc.sync.dma_start(ones_sb, ones64[:, :])
            wo_sb = cpool.tile([E, D], BF, name="wo_sb")
            nc.sync.dma_start(wo_sb, wo2[:, :])

            # ---- persistent activations -----------------------------------
            qT = pers.tile([E, BT], BF, name="qT")
            kT = pers.tile([E, BT], BF, name="kT")
            v = pers.tile([128, NKT, HPC, HD + 1], BF, name="v")
            ctxT = pers.tile([128, BT], BF, name="ctxT")
            nc.sync.dma_start(v[:, :, :, HD], ones64[:, : NKT * HPC])

            # ---- phase A: QKV projections ---------------------------------
            for t in range(NTCH):
                cols = bass.ts(t, TCH)
                xt = work.tile([128, D // 128, TCH], BF, name="xt", tag="xt", bufs=2)
                nc.sync.dma_start(
                    xt, xT[:, cols].rearrange("(n p) m -> p n m", p=128)
                )
                for w_sb, b_sb, dst in (
                    (wq_sb, bq_sb, qT),
                    (wk_sb, bk_sb, kT),
                    (wv_sb, bv_sb, None),
                ):
                    ps = psum.tile([128, TCH], F32, name="ps_mm", tag="mm", bufs=2)
                    for d in range(D // 128):
                        nc.tensor.matmul(
                            ps,
                            lhsT=w_sb[:, d, :],
                            rhs=xt[:, d, :],
                            start=(d == 0),
                            stop=(d == D // 128 - 1),
                        )
                    if dst is not None:
                        nc.scalar.activation(
                            dst[:, cols], ps, AF.Identity, bias=b_sb, scale=1.0
                        )
                    else:
                        vt = work.tile([128, TCH], BF, name="vt", tag="vt", bufs=2)
                        nc.scalar.activation(vt, ps, AF.Identity, bias=b_sb, scale=1.0)
                        # transpose v back to [token, dim] layout, 128 at a time
                        for i in range(TCH // 128):
                            kt_idx = t * (TCH // 128) + i
                            tp = psum.tile(
                                [128, 128], BF, name="tp", tag="s", bufs=2
                            )
                            nc.tensor.transpose(tp, vt[:, bass.ts(i, 128)], ident)
                            for h in range(HPC):
                                nc.vector.tensor_copy(
                                    v[:, kt_idx, h, 0:HD], tp[:, bass.ts(h, HD)]
                                )

            # ---- phases B+C: attention + output projection, pipelined ------
            # Per k-tile, both heads' S^T matmuls are row-tiled (T0/T8) so
            # they run concurrently on the PE; both land in one [128, 1024]
            # PSUM tile. Exp runs on the scalar engine except every 4th
            # k-tile, which uses a Schraudolph bit-trick exp on the DVE
            # (bits16(e^x) ~= round(x*128/ln2 + 16233) viewed as bf16).
            # ctx matmuls are pipelined one 2-k-tile block behind.
            #
            # Softmax normalization is deferred: per (h, qc) the ctx psum is
            # staged to SBUF (cs, bf16) and its denominator row is DMA'd into
            # one den_all row; per *batch* a single reciprocal serves all 8
            # rows, and the scale-multiplies + output projection of batch b
            # run interleaved with batch b+1's attention so the PE never
            # idles long enough to re-throttle.
            EXP_DVE_EVERY = 4       # every 4th k-tile's exp on DVE
            EXP_A = 128.0 / float(np.log(2.0)) / 8.0   # folds the 1/8 scale
            EXP_B = 16249.0  # calibrated for zero mean bias vs exact exp
            BLK = 2
            NBLK = KTB // BLK
            NQC = T // TCH          # q chunks per batch
            NST = B * NQC           # global q-chunk steps
            den_alls = {}
            css = {}

            def attn_qchunk(b, qc):
                q0 = b * T + qc * TCH
                cps = []
                for h in range(HPC):
                    cp = psum.tile(
                        [HD + 1, TCH], F32, name=f"cp{h}", tag=f"ctx{h}", bufs=1
                    )
                    cps.append(cp)
                pts = {}
                for blk in range(NBLK + 1):
                    if blk < NBLK:
                        for kt in range(blk * BLK, (blk + 1) * BLK):
                            k0 = b * T + kt * 128
                            sp = psum.tile(
                                [128, HPC * TCH], F32, name="sp", tag="s", bufs=2
                            )
                            for h in range(HPC):
                                he = bass.ts(h, HD)
                                nc.tensor.matmul(
                                    sp[:, bass.ts(h, TCH)],
                                    lhsT=kT[he, k0 : k0 + 128],
                                    rhs=qT[he, q0 : q0 + TCH],
                                    start=True,
                                    stop=True,
                                )
                            pt = work.tile(
                                [128, HPC * TCH], BF, name="pt", tag="pt",
                                bufs=8,
                            )
                            if kt % EXP_DVE_EVERY == EXP_DVE_EVERY - 1:
                                nc.vector.tensor_scalar(
                                    pt.bitcast(mybir.dt.int16), sp,
                                    EXP_A, EXP_B,
                                    op0=mybir.AluOpType.mult,
                                    op1=mybir.AluOpType.add,
                                )
                            else:
                                nc.scalar.activation(
                                    pt, sp, AF.Exp, scale=1.0 / 8.0
                                )
                            pts[kt] = pt
                    if blk > 0:
                        for kt in range((blk - 1) * BLK, blk * BLK):
                            for h in range(HPC):
                                nc.tensor.matmul(
                                    cps[h],
                                    lhsT=v[:, b * KTB + kt, h, :],
                                    rhs=pts[kt][:, bass.ts(h, TCH)],
                                    start=(kt == 0),
                                    stop=(kt == KTB - 1),
                                    skip_group_check=True,
                                )
                # stage ctx+den to SBUF (frees psum); den rows collect into
                # a 2-step window tile via partition-shifting DMA so one
                # reciprocal serves 4 rows.
                s = b * NQC + qc
                for h in range(HPC):
                    cs = work.tile(
                        [HD + 1, TCH], BF, name="cs", tag="cs", bufs=8
                    )
                    nc.vector.tensor_copy(cs, cps[h])
                    nc.sync.dma_start(
                        den_alls[s // 2][(s % 2) * HPC + h :
                                         (s % 2) * HPC + h + 1, :],
                        cs[HD : HD + 1, :],
                    )
                    css[(s, h)] = cs

            def norm_recip(w):
                nc.vector.reciprocal(den_alls[w], den_alls[w])
                # LDWEIGHTS needs 32-aligned partitions: shift each recip row
                # back to partition 0 so the broadcast matmuls can read it.
                for i in range(2 * HPC):
                    nc.sync.dma_start(
                        rdens[w][0:1, i, :], den_alls[w][i : i + 1, :]
                    )

            def norm_mult(s, h):
                q0 = s * TCH
                i = (s % 2) * HPC + h
                cs = css.pop((s, h))
                rb = psum.tile([HD, TCH], F32, name="rb", tag="mm", bufs=2)
                nc.tensor.matmul(
                    rb,
                    lhsT=ones_sb[0:1, :],
                    rhs=rdens[s // 2][0:1, i, :],
                    start=True,
                    stop=True,
                )
                if h == 0:
                    nc.vector.tensor_tensor(
                        ctxT[0:HD, q0 : q0 + TCH],
                        cs[0:HD, :],
                        rb,
                        op=mybir.AluOpType.mult,
                    )
                else:
                    # h1 lives on partitions 64-127 of ctxT; engines can't
                    # shift partitions, so stage and DMA.
                    ctxs = work.tile(
                        [HD, TCH], BF, name="ctxs", tag="ctxs", bufs=2
                    )
                    nc.vector.tensor_tensor(
                        ctxs,
                        cs[0:HD, :],
                        rb,
                        op=mybir.AluOpType.mult,
                    )
                    nc.sync.dma_start(ctxT[HD:128, q0 : q0 + TCH], ctxs)

            def out_proj_tile(tt):
                # ctxT stacks both heads on partitions -> one K=128 matmul
                # per (token tile, out chunk). The two chunks' po copies go
                # to ACT and DVE, then one 256KB DMA writes the full row.
                trows = bass.ts(tt, 128)
                ob = work.tile([128, D], BF, name="ob", tag="ob", bufs=3)
                for nch in range(D // TCH):
                    po = psum.tile([128, TCH], F32, name="po", tag="mm", bufs=2)
                    nc.tensor.matmul(
                        po,
                        lhsT=ctxT[:, trows],
                        rhs=wo_sb[:, bass.ts(nch, TCH)],
                        start=True,
                        stop=True,
                    )
                    if nch % 2 == 0:
                        nc.scalar.activation(
                            ob[:, bass.ts(nch, TCH)], po, AF.Copy
                        )
                    else:
                        nc.vector.tensor_copy(ob[:, bass.ts(nch, TCH)], po)
                nc.sync.dma_start(out[trows, :], ob)

            rdens = {}
            for w in range(NST // 2):
                den_alls[w] = work.tile(
                    [2 * HPC, TCH], BF, name=f"den_all{w}", tag=f"den{w}",
                    bufs=1,
                )
                rdens[w] = work.tile(
                    [1, 2 * HPC, TCH], BF, name=f"rden{w}", tag=f"rden{w}",
                    bufs=1,
                )

            # sliding pipeline over 8 global q-chunk steps: attention leads,
            # normalize + output projection trail by 2 steps (their recip
            # window completes at every odd step).
            TPS = TCH // 128        # token tiles per step
            def norm_and_proj(s):
                for h in range(HPC):
                    norm_mult(s, h)
                for tt in range(s * TPS, (s + 1) * TPS):
                    out_proj_tile(tt)

            for s in range(NST):
                attn_qchunk(s // NQC, s % NQC)
                if s % 2 == 1:
                    norm_recip(s // 2)
                if s >= 2:
                    norm_and_proj(s - 2)
            for s in range(NST - 2, NST):
                norm_and_proj(s)

    _split_matmul_waits(nc)
    return nc


def _split_matmul_waits(nc):
    """This walrus allows only one sync wait per engine instruction (and none
    on fp32/f32r InstMatmult, whose embedded S3_LW carries the wait slot).
    Move excess waits onto InstEventSemaphore instructions (capacity 2)
    inserted just before the owner in the same engine stream — sequencer
    dispatch is in-order, so semantics are unchanged."""
    ctr = 0
    for f in nc.m.functions:
        for blk in f.blocks:
            out = []
            for inst in blk.instructions:
                si = inst.sync_info
                if (
                    si is not None
                    and not isinstance(inst, mybir.InstEventSemaphore)
                    and len(si.on_wait) > 1
                ):
                    waits = list(si.on_wait)
                    keep = [waits.pop(0)]
                    for i in range(0, len(waits), 2):
                        ev = mybir.InstEventSemaphore(name=f"I-exwait-{ctr}")
                        ctr += 1
                        ev.engine = inst.engine
                        ev.sync_info = mybir.SyncInfo(
                            on_wait=waits[i : i + 2], on_update=[]
                        )
                        nc.register_instruction(ev)
                        out.append(ev)
                    si.on_wait = keep
                out.append(inst)
            blk.instructions[:] = out


_CACHE = {}


def _get_nc():
    if "nc" not in _CACHE:
        _CACHE["nc"] = build_nc()
    return _CACHE["nc"]


def make_in_maps(x, w_qkv, b_qkv, w_out):
    bf = ml_dtypes.bfloat16
    x = np.ascontiguousarray(np.asarray(x, np.float32)).reshape(BT, D)
    w_qkv = np.asarray(w_qkv, np.float32)
    b_qkv = np.asarray(b_qkv, np.float32)
    w_out = np.asarray(w_out, np.float32)

    xT = np.ascontiguousarray(x.T).astype(bf)  # [D, BT]
    wq, wk, wv = w_qkv[0:D], w_qkv[D : 2 * D], w_qkv[2 * D : 3 * D]
    bqs, bks, bvs = b_qkv[0:D], b_qkv[D : 2 * D], b_qkv[2 * D : 3 * D]
    ones = np.ones((128, HD), bf)
    ident = np.eye(128, dtype=np.float32).astype(bf)

    in_maps = []
    for c in range(NCORES):
        rs = slice(E * c, E * (c + 1))
        # wo2[r, o] = w_out[o, E*c + r]  (r = h*64+hd packs both heads on K)
        wo_c = np.ascontiguousarray(w_out[:, rs].T).astype(bf)
        in_maps.append(
            {
                "xT": xT,
                "wqT": np.ascontiguousarray(wq[rs].T).astype(bf),
                "wkT": np.ascontiguousarray(wk[rs].T).astype(bf),
                "wvT": np.ascontiguousarray(wv[rs].T).astype(bf),
                "bq": np.ascontiguousarray(bqs[rs])[:, None],
                "bk": np.ascontiguousarray(bks[rs])[:, None],
                "bv": np.ascontiguousarray(bvs[rs])[:, None],
                "wo2": wo_c,
                "ones64": ones,
                "ident128": ident,
            }
        )
    return in_maps


def _combine(results, b_out):
    acc = results[0]["out"].astype(np.float32)
    for r in results[1:]:
        acc += r["out"].astype(np.float32)
    acc += np.asarray(b_out, np.float32)[None, :]
    return acc.reshape(B, T, D)


def kernel(x, w_qkv, b_qkv, w_out, b_out):
    in_maps = make_in_maps(x, w_qkv, b_qkv, w_out)
    res = run_bass_kernel_spmd(_get_nc(), in_maps, core_ids=list(range(NCORES)))
    return _combine(res.results, b_out)


def kernel_traced(x, w_qkv, b_qkv, w_out, b_out):
    """Like kernel() but profiles the run; returns (output, exec_time_ns)."""
    in_maps = make_in_maps(x, w_qkv, b_qkv, w_out)
    res = run_bass_kernel_spmd(
        _get_nc(), in_maps, core_ids=list(range(NCORES)), trace=True
    )
    return _combine(res.results, b_out), res.exec_time_ns


# revision 21
# speedup vs baseline: 1.5995x; 1.0437x over previous
# Multi-head attention (B=2, T=2048, D=1024, H=16) on 8 TRN2 NeuronCores.
#
# Sharding: tensor-parallel over heads. Each core owns 2 heads (a 128-wide
# slice of the hidden dim): it computes its q/k/v projection slice, full
# attention for its 4 (batch, head) pairs, and a partial output projection
# over its slice of the contraction. The 8 partial outputs are summed on the
# host (the TP all-reduce, done as part of unsharding), plus the output bias.
#
# All matmul operands are bf16 (PSUM accumulation stays fp32): rel tolerance
# is 2e-2 and bf16 keeps us ~2.5e-3, while halving DMA/SBUF traffic and
# letting weight loads overlap matmul streaming.
#
# Layouts (per core):
#   xT   [D=1024, B*T=4096]   x transposed so the contraction dim is on SBUF
#                             partitions for the projection matmuls.
#   qT/kT [128, 4096]         head-dim on partitions (2 heads stacked), token
#                             on free axis -> S^T tiles come out of the PE
#                             directly with softmax's reduction on the free
#                             axis of P^T's consumer.
#   v    [128tok, 32kt, 2h, 65]  natural [token, dim] layout per k-tile with a
#                             ones column appended: the ctx matmul then yields
#                             the softmax denominator for free in row 64.
#   ctxT [128, 4096]          both heads' normalized context stacked on
#                             partitions (h0 rows 0-63, h1 rows 64-127) so the
#                             output projection contracts K=128 in one matmul
#                             per tile.
import sys
import types

import numpy as np


def _install_ntff_hook_shim():
    """This image's `antenv` lacks `axon_hooks`, which bass_utils imports
    unconditionally when tracing is requested (e.g. BASS_TRACE=1). Provide
    the module and register the ctypes NTFF hook the way trn_boot would."""
    try:
        import antenv.axon_hooks  # noqa: F401

        return
    except ImportError:
        pass
    try:
        import antenv
    except ImportError:
        return
    mod = types.ModuleType("antenv.axon_hooks")
    _h = [None]
    mod.set_axon_ntff_profile_hook = lambda h: _h.__setitem__(0, h)
    mod.get_axon_ntff_profile_hook = lambda: _h[0]
    sys.modules["antenv.axon_hooks"] = mod
    antenv.axon_hooks = mod
    try:
        from trn_agent_boot.trn_boot import _ntff_profile_via_ctypes

        mod.set_axon_ntff_profile_hook(
            _ntff_profile_via_ctypes("/opt/axon/libaxon_pjrt.so")
        )
    except Exception:
        pass


_install_ntff_hook_shim()

import ml_dtypes

import concourse.bass as bass
import concourse.mybir as mybir
from concourse.bass_utils import run_bass_kernel_spmd
from concourse.tile import TileContext

B, T, D, H = 2, 2048, 1024, 16
HD = D // H          # 64
NCORES = 8
BT = B * T           # 4096
E = D // NCORES      # 128 = per-core slice of hidden dim (2 heads)
HPC = E // HD        # 2 heads per core

F32 = mybir.dt.float32
BF = mybir.dt.bfloat16
AF = mybir.ActivationFunctionType

TCH = 512            # token chunk for projections / q chunks
NTCH = BT // TCH     # 8
NKT = BT // 128      # 32 token tiles of 128
KTB = T // 128       # 16 k-tiles per batch


def build_nc():
    nc = bass.Bass()

    xT = nc.dram_tensor("xT", [D, BT], BF, kind="ExternalInput")
    wqT = nc.dram_tensor("wqT", [D, E], BF, kind="ExternalInput")
    wkT = nc.dram_tensor("wkT", [D, E], BF, kind="ExternalInput")
    wvT = nc.dram_tensor("wvT", [D, E], BF, kind="ExternalInput")
    bq = nc.dram_tensor("bq", [E, 1], F32, kind="ExternalInput")
    bk = nc.dram_tensor("bk", [E, 1], F32, kind="ExternalInput")
    bv = nc.dram_tensor("bv", [E, 1], F32, kind="ExternalInput")
    wo2 = nc.dram_tensor("wo2", [E, D], BF, kind="ExternalInput")
    ones64 = nc.dram_tensor("ones64", [128, HD], BF, kind="ExternalInput")
    ident128 = nc.dram_tensor("ident128", [128, 128], BF, kind="ExternalInput")
    out = nc.dram_tensor("out", [BT, D], BF, kind="ExternalOutput")

    with TileContext(nc) as tc:
        with (
            nc.allow_low_precision(reason="bf16 matmuls are deliberate"),
            tc.tile_pool(name="const", bufs=1) as cpool,
            tc.tile_pool(name="pers", bufs=1) as pers,
            tc.tile_pool(name="work", bufs=2) as work,
            tc.tile_pool(name="psum", bufs=2, space="PSUM") as psum,
        ):
            # ---- constants -------------------------------------------------
            # DMA order matters at startup: wq + x chunks 0-1 first so the
            # first projection matmuls start ASAP; wo_sb (first needed by the
            # out-projection much later) goes last.
            wq_sb = cpool.tile([128, D // 128, E], BF, name="wq_sb")
            wk_sb = cpool.tile([128, D // 128, E], BF, name="wk_sb")
            wv_sb = cpool.tile([128, D // 128, E], BF, name="wv_sb")
            xts = [
                work.tile([128, D // 128, TCH], BF, name="xt", tag="xt", bufs=3)
                for _ in range(NTCH)
            ]
            nc.sync.dma_start(wq_sb, wqT.rearrange("(n p) m -> p n m", p=128))
            nc.sync.dma_start(
                xts[0], xT[:, bass.ts(0, TCH)].rearrange("(n p) m -> p n m", p=128)
            )
            bq_sb = cpool.tile([E, 1], F32, name="bq_sb")
            bk_sb = cpool.tile([E, 1], F32, name="bk_sb")
            bv_sb = cpool.tile([E, 1], F32, name="bv_sb")
            nc.sync.dma_start(bq_sb, bq[:, :])
            nc.sync.dma_start(bk_sb, bk[:, :])
            nc.sync.dma_start(bv_sb, bv[:, :])
            nc.sync.dma_start(wk_sb, wkT.rearrange("(n p) m -> p n m", p=128))
            nc.sync.dma_start(wv_sb, wvT.rearrange("(n p) m -> p n m", p=128))
            nc.sync.dma_start(
                xts[1], xT[:, bass.ts(1, TCH)].rearrange("(n p) m -> p n m", p=128)
            )
            ident = cpool.tile([128, 128], BF, name="ident")
            nc.sync.dma_start(ident, ident128[:, :])
            ones_sb = cpool.tile([128, HD], BF, name="ones_sb")
            nc.sync.dma_start(ones_sb, ones64[:, :])
            wo_sb = cpool.tile([E, D], BF, name="wo_sb")
            nc.sync.dma_start(wo_sb, wo2[:, :])

            # ---- persistent activations -----------------------------------
            qT = pers.tile([E, BT], BF, name="qT")
            kT = pers.tile([E, BT], BF, name="kT")
            v = pers.tile([128, NKT, HPC, HD + 1], BF, name="v")
            ctxT = pers.tile([128, BT], BF, name="ctxT")
            nc.sync.dma_start(v[:, :, :, HD], ones64[:, : NKT * HPC])

            # ---- phase A: QKV projections ---------------------------------
            for t in range(NTCH):
                cols = bass.ts(t, TCH)
                xt = xts[t]
                if t > 1:
                    nc.sync.dma_start(
                        xt, xT[:, cols].rearrange("(n p) m -> p n m", p=128)
                    )
                for w_sb, b_sb, dst in (
                    (wq_sb, bq_sb, qT),
                    (wk_sb, bk_sb, kT),
                    (wv_sb, bv_sb, None),
                ):
                    ps = psum.tile([128, TCH], F32, name="ps_mm", tag="mm", bufs=2)
                    for d in range(D // 128):
                        nc.tensor.matmul(
                            ps,
                            lhsT=w_sb[:, d, :],
                            rhs=xt[:, d, :],
                            start=(d == 0),
                            stop=(d == D // 128 - 1),
                        )
                    if dst is not None:
                        nc.scalar.activation(
                            dst[:, cols], ps, AF.Identity, bias=b_sb, scale=1.0
                        )
                    else:
                        vt = work.tile([128, TCH], BF, name="vt", tag="vt", bufs=2)
                        nc.scalar.activation(vt, ps, AF.Identity, bias=b_sb, scale=1.0)
                        # transpose v back to [token, dim] layout, 128 at a time
                        for i in range(TCH // 128):
                            kt_idx = t * (TCH // 128) + i
                            tp = psum.tile(
                                [128, 128], BF, name="tp", tag="s", bufs=2
                            )
                            nc.tensor.transpose(tp, vt[:, bass.ts(i, 128)], ident)
                            for h in range(HPC):
                                nc.vector.tensor_copy(
                                    v[:, kt_idx, h, 0:HD], tp[:, bass.ts(h, HD)]
                                )

            # ---- phases B+C: attention + output projection, pipelined ------
            # Per k-tile, both heads' S^T matmuls are row-tiled (T0/T8) so
            # they run concurrently on the PE; both land in one [128, 1024]
            # PSUM tile. Exp runs on the scalar engine except every 4th
            # k-tile, which uses a Schraudolph bit-trick exp on the DVE
            # (bits16(e^x) ~= round(x*128/ln2 + 16233) viewed as bf16).
            # ctx matmuls are pipelined one 2-k-tile block behind.
            #
            # Softmax normalization is deferred: per (h, qc) the ctx psum is
            # staged to SBUF (cs, bf16) and its denominator row is DMA'd into
            # one den_all row; per *batch* a single reciprocal serves all 8
            # rows, and the scale-multiplies + output projection of batch b
            # run interleaved with batch b+1's attention so the PE never
            # idles long enough to re-throttle.
            EXP_DVE_EVERY = 4       # every 4th k-tile's exp on DVE
            EXP_A = 128.0 / float(np.log(2.0)) / 8.0   # folds the 1/8 scale
            EXP_B = 16249.0  # calibrated for zero mean bias vs exact exp
            BLK = 2
            NBLK = KTB // BLK
            NQC = T // TCH          # q chunks per batch
            NST = B * NQC           # global q-chunk steps
            den_alls = {}
            css = {}

            def attn_qchunk(b, qc, weave=()):
                # `weave`: list of closures (trailing normalize / projection
                # work) emitted one per k-tile block so their PE/DVE/ACT ops
                # interleave with this chunk's attention instead of bunching
                # up at a phase boundary.
                weave = list(weave)
                q0 = b * T + qc * TCH
                cps = []
                for h in range(HPC):
                    cp = psum.tile(
                        [HD + 1, TCH], F32, name=f"cp{h}", tag=f"ctx{h}", bufs=1
                    )
                    cps.append(cp)
                pts = {}
                for blk in range(NBLK + 1):
                    if weave:
                        weave.pop(0)()
                    if blk < NBLK:
                        for kt in range(blk * BLK, (blk + 1) * BLK):
                            k0 = b * T + kt * 128
                            sp = psum.tile(
                                [128, HPC * TCH], F32, name="sp", tag="s", bufs=2
                            )
                            for h in range(HPC):
                                he = bass.ts(h, HD)
                                nc.tensor.matmul(
                                    sp[:, bass.ts(h, TCH)],
                                    lhsT=kT[he, k0 : k0 + 128],
                                    rhs=qT[he, q0 : q0 + TCH],
                                    start=True,
                                    stop=True,
                                )
                            pt = work.tile(
                                [128, HPC * TCH], BF, name="pt", tag="pt",
                                bufs=8,
                            )
                            if kt % EXP_DVE_EVERY == EXP_DVE_EVERY - 1:
                                nc.vector.tensor_scalar(
                                    pt.bitcast(mybir.dt.int16), sp,
                                    EXP_A, EXP_B,
                                    op0=mybir.AluOpType.mult,
                                    op1=mybir.AluOpType.add,
                                )
                            else:
                                nc.scalar.activation(
                                    pt, sp, AF.Exp, scale=1.0 / 8.0
                                )
                            pts[kt] = pt
                    if blk > 0:
                        for kt in range((blk - 1) * BLK, blk * BLK):
                            for h in range(HPC):
                                nc.tensor.matmul(
                                    cps[h],
                                    lhsT=v[:, b * KTB + kt, h, :],
                                    rhs=pts[kt][:, bass.ts(h, TCH)],
                                    start=(kt == 0),
                                    stop=(kt == KTB - 1),
                                    skip_group_check=True,
                                )
                for op in weave:
                    op()
                # stage ctx+den to SBUF (frees psum); den rows collect into
                # the group window tile at 32-aligned partitions (so the
                # broadcast matmuls can read the recip'd rows directly) via
                # partition-shifting DMA. One head's copy goes through the
                # scalar engine to keep the DVE queue short.
                s = b * NQC + qc
                for h in range(HPC):
                    cs = work.tile(
                        [HD + 1, TCH], BF, name="cs", tag="cs", bufs=8
                    )
                    if h == 0:
                        nc.scalar.activation(cs, cps[h], AF.Copy)
                    else:
                        nc.vector.tensor_copy(cs, cps[h])
                    r = den_row(s, h)
                    nc.sync.dma_start(
                        den_alls[GROUP[s]][r : r + 1, :],
                        cs[HD : HD + 1, :],
                    )
                    css[(s, h)] = cs

            def norm_recip(g):
                nc.vector.reciprocal(den_alls[g], den_alls[g])

            def norm_mult(s, h):
                q0 = s * TCH
                r = den_row(s, h)
                cs = css.pop((s, h))
                rb = psum.tile([HD, TCH], F32, name="rb", tag="mm", bufs=2)
                nc.tensor.matmul(
                    rb,
                    lhsT=ones_sb[r : r + 1, :],
                    rhs=den_alls[GROUP[s]][r : r + 1, :],
                    start=True,
                    stop=True,
                )
                if h == 0:
                    nc.vector.tensor_tensor(
                        ctxT[0:HD, q0 : q0 + TCH],
                        cs[0:HD, :],
                        rb,
                        op=mybir.AluOpType.mult,
                    )
                else:
                    # h1 lives on partitions 64-127 of ctxT; engines can't
                    # shift partitions, so stage and DMA.
                    ctxs = work.tile(
                        [HD, TCH], BF, name="ctxs", tag="ctxs", bufs=2
                    )
                    nc.vector.tensor_tensor(
                        ctxs,
                        cs[0:HD, :],
                        rb,
                        op=mybir.AluOpType.mult,
                    )
                    nc.sync.dma_start(ctxT[HD:128, q0 : q0 + TCH], ctxs)

            def out_proj_tile(tt):
                # ctxT stacks both heads on partitions -> one K=128 matmul
                # per (token tile, out chunk). The two chunks' po copies go
                # to ACT and DVE, then one 256KB DMA writes the full row.
                trows = bass.ts(tt, 128)
                ob = work.tile([128, D], BF, name="ob", tag="ob", bufs=3)
                for nch in range(D // TCH):
                    po = psum.tile([128, TCH], F32, name="po", tag="mm", bufs=2)
                    nc.tensor.matmul(
                        po,
                        lhsT=ctxT[:, trows],
                        rhs=wo_sb[:, bass.ts(nch, TCH)],
                        start=True,
                        stop=True,
                    )
                    if nch % 2 == 0:
                        nc.scalar.activation(
                            ob[:, bass.ts(nch, TCH)], po, AF.Copy
                        )
                    else:
                        nc.vector.tensor_copy(ob[:, bass.ts(nch, TCH)], po)
                nc.sync.dma_start(out[trows, :], ob)

            # each step has its own den window (rows at partitions {0, 32} —
            # matmul operands may only start at 0/32/64, and quadrant 3 is
            # off limits). recip(s) weaves into attn(s+1); norm_mult(s) into
            # attn(s+2), a full step after its recip, so the PE's in-order
            # queue never stalls on a freshly-queued reciprocal.
            GROUP = list(range(NST))

            def den_row(s, h):
                return 32 * h

            for g in range(NST):
                den_alls[g] = work.tile(
                    [33, TCH], BF, name=f"den_all{g}", tag=f"den{g}", bufs=1
                )
                nc.vector.memset(den_alls[g], 1.0)

            # sliding pipeline over 8 global q-chunk steps: attention leads,
            # normalize + output projection trail by 2 steps, woven one op
            # per k-tile block of the leading attention chunk.
            TPS = TCH // 128        # token tiles per step

            def trailing_ops(s):
                ops = []
                if s >= 1:
                    ops.append(lambda g=s - 1: norm_recip(g))
                if s >= 2:
                    p = s - 2
                    for h in range(HPC):
                        ops.append(lambda p=p, h=h: norm_mult(p, h))
                    for tt in range(p * TPS, (p + 1) * TPS):
                        ops.append(lambda tt=tt: out_proj_tile(tt))
                return ops

            for s in range(NST):
                attn_qchunk(s // NQC, s % NQC, weave=trailing_ops(s))
            norm_recip(NST - 1)
            for s in range(NST - 2, NST):
                for h in range(HPC):
                    norm_mult(s, h)
                for tt in range(s * TPS, (s + 1) * TPS):
                    out_proj_tile(tt)

    _split_matmul_waits(nc)
    return nc


def _split_matmul_waits(nc):
    """This walrus allows only one sync wait per engine instruction (and none
    on fp32/f32r InstMatmult, whose embedded S3_LW carries the wait slot).
    Move excess waits onto InstEventSemaphore instructions (capacity 2)
    inserted just before the owner in the same engine stream — sequencer
    dispatch is in-order, so semantics are unchanged."""
    ctr = 0
    for f in nc.m.functions:
        for blk in f.blocks:
            out = []
            for inst in blk.instructions:
                si = inst.sync_info
                if (
                    si is not None
                    and not isinstance(inst, mybir.InstEventSemaphore)
                    and len(si.on_wait) > 1
                ):
                    waits = list(si.on_wait)
                    keep = [waits.pop(0)]
                    for i in range(0, len(waits), 2):
                        ev = mybir.InstEventSemaphore(name=f"I-exwait-{ctr}")
                        ctr += 1
                        ev.engine = inst.engine
                        ev.sync_info = mybir.SyncInfo(
                            on_wait=waits[i : i + 2], on_update=[]
                        )
                        nc.register_instruction(ev)
                        out.append(ev)
                    si.on_wait = keep
                out.append(inst)
            blk.instructions[:] = out


_CACHE = {}


def _get_nc():
    if "nc" not in _CACHE:
        _CACHE["nc"] = build_nc()
    return _CACHE["nc"]


def make_in_maps(x, w_qkv, b_qkv, w_out):
    bf = ml_dtypes.bfloat16
    x = np.ascontiguousarray(np.asarray(x, np.float32)).reshape(BT, D)
    w_qkv = np.asarray(w_qkv, np.float32)
    b_qkv = np.asarray(b_qkv, np.float32)
    w_out = np.asarray(w_out, np.float32)

    xT = np.ascontiguousarray(x.T).astype(bf)  # [D, BT]
    wq, wk, wv = w_qkv[0:D], w_qkv[D : 2 * D], w_qkv[2 * D : 3 * D]
    bqs, bks, bvs = b_qkv[0:D], b_qkv[D : 2 * D], b_qkv[2 * D : 3 * D]
    ones = np.ones((128, HD), bf)
    ident = np.eye(128, dtype=np.float32).astype(bf)

    in_maps = []
    for c in range(NCORES):
        rs = slice(E * c, E * (c + 1))
        # wo2[r, o] = w_out[o, E*c + r]  (r = h*64+hd packs both heads on K)
        wo_c = np.ascontiguousarray(w_out[:, rs].T).astype(bf)
        in_maps.append(
            {
                "xT": xT,
                "wqT": np.ascontiguousarray(wq[rs].T).astype(bf),
                "wkT": np.ascontiguousarray(wk[rs].T).astype(bf),
                "wvT": np.ascontiguousarray(wv[rs].T).astype(bf),
                "bq": np.ascontiguousarray(bqs[rs])[:, None],
                "bk": np.ascontiguousarray(bks[rs])[:, None],
                "bv": np.ascontiguousarray(bvs[rs])[:, None],
                "wo2": wo_c,
                "ones64": ones,
                "ident128": ident,
            }
        )
    return in_maps


def _combine(results, b_out):
    acc = results[0]["out"].astype(np.float32)
    for r in results[1:]:
        acc += r["out"].astype(np.float32)
    acc += np.asarray(b_out, np.float32)[None, :]
    return acc.reshape(B, T, D)


def kernel(x, w_qkv, b_qkv, w_out, b_out):
    in_maps = make_in_maps(x, w_qkv, b_qkv, w_out)
    res = run_bass_kernel_spmd(_get_nc(), in_maps, core_ids=list(range(NCORES)))
    return _combine(res.results, b_out)


def kernel_traced(x, w_qkv, b_qkv, w_out, b_out):
    """Like kernel() but profiles the run; returns (output, exec_time_ns)."""
    in_maps = make_in_maps(x, w_qkv, b_qkv, w_out)
    res = run_bass_kernel_spmd(
        _get_nc(), in_maps, core_ids=list(range(NCORES)), trace=True
    )
    return _combine(res.results, b_out), res.exec_time_ns
